# revision 11
# baseline (speedup 1.0000x reference)
"""Trainium2 Bass kernel for windowed multi-head attention with a dynamic
position-bias MLP (CrossFormer-style), data-parallel over windows on 8 cores.

Math per window (N=256 tokens, C=512 dim, H=8 heads, hd=64):
    qkv = x @ qkv_w + qkv_b ; q scaled by hd**-0.5
    attn = softmax(q @ k^T + rpb) ; out = (attn @ v) @ proj_w + proj_b
where rpb[h] = pos[rel_idx] and pos = MLP(biases) is a tiny 4-layer MLP
(LayerNorm + ReLU) applied to the 961 relative-offset rows, shared by all
windows.

Layout strategy on each NeuronCore (32 windows/core):
  - x is PE-transposed to x^T (channels on partitions).
  - q^T/k^T are produced channel-major ([c, t]); v token-major ([t, c]).
  - S^T = k^T.T @ q^T computed per head with 2-head row-packing (K=64).
  - P^T = exp(0.125*S^T) * exp_rpb^T   (softmax without max-subtraction:
    logits are O(1) by construction, exp cannot overflow; rpb enters as a
    multiplicative exp table).
  - O^T = v_aug.T @ P^T with a ones-column in v_aug producing the softmax
    denominator as row 64; rows are divided by it during evacuation.
  - y = proj applied with O^T directly as the stationary operand.
  - exp_rpb^T is gathered once per launch from DRAM with a structured
    (block-Toeplitz) access pattern in a reversed-within-16 row order (the
    only DMA-legal order), then fixed up with permutation-transposes.

All matmuls run as float32r (full fp32 data; ~1e-3 matmul rounding, 4x
faster than fp32 mode on the PE).
"""

import os
import sys

if "axon" not in os.environ.get("JAX_PLATFORMS", ""):
    os.environ["JAX_PLATFORMS"] = "axon"

for _p in (
    "/root/.axon_site",
    "/root/.axon_site/_ro/trn_rl_repo",
    "/root/.axon_site/_ro/pypackages",
    "/opt/trn_rl_repo",
):
    if os.path.isdir(_p) and _p not in sys.path:
        sys.path.append(_p)

import numpy as np

import concourse.bass as bass
import concourse.bacc as bacc
import concourse.mybir as mybir
import concourse.tile as tile
from concourse.bass_utils import run_bass_kernel_spmd

F32 = mybir.dt.float32
F32R = mybir.dt.float32r
EXP = mybir.ActivationFunctionType.Exp
SQRT = mybir.ActivationFunctionType.Sqrt
COPY = mybir.ActivationFunctionType.Copy
ADD = mybir.AluOpType.add
SUB = mybir.AluOpType.subtract
MULT = mybir.AluOpType.mult
MAX = mybir.AluOpType.max

B, N, C = 256, 256, 512
H, HD = 8, 64
PD, L = 32, 961  # pos-MLP width, (2*16-1)**2 offset rows
GH = GW = 16
NCORES = 8
WPC = B // NCORES  # windows per core
SCALE = HD ** -0.5
LN_EPS = 1e-5


def _host_consts():
    ident = np.eye(128, dtype=np.float32)
    sigma = np.array([(i // 16) * 16 + (15 - i % 16) for i in range(128)])
    sigperm = np.zeros((128, 128), np.float32)
    for i in range(128):
        sigperm[i, sigma[i]] = 1.0
    ones = np.ones((128, 128), np.float32)
    return ident, sigperm, ones


def build_program(wpc=WPC, repeat=1):
    """Build the SPMD Bass program for one core handling `wpc` windows.

    repeat>1 wraps the steady-state window loop in a hardware For loop for
    wall-clock timing (the computation is idempotent)."""
    nc = bacc.Bacc("TRN2", num_devices=NCORES)
    T = wpc * N  # tokens per core

    x_d = nc.dram_tensor("x", [T, C], F32R, kind="ExternalInput")
    qkvw_d = nc.dram_tensor("qkv_w", [C, 3 * C], F32R, kind="ExternalInput")
    qkvb_d = nc.dram_tensor("qkv_b", [3 * C], F32, kind="ExternalInput")
    projw_d = nc.dram_tensor("proj_w", [C, C], F32R, kind="ExternalInput")
    projb_d = nc.dram_tensor("proj_b", [C], F32, kind="ExternalInput")
    pw_d = [
        nc.dram_tensor("p1_w", [2, PD], F32R, kind="ExternalInput"),
        nc.dram_tensor("p2_w", [PD, PD], F32R, kind="ExternalInput"),
        nc.dram_tensor("p3_w", [PD, PD], F32R, kind="ExternalInput"),
    ]
    pb_d = [
        nc.dram_tensor("p1_b", [PD], F32, kind="ExternalInput"),
        nc.dram_tensor("p2_b", [PD], F32, kind="ExternalInput"),
        nc.dram_tensor("p3_b", [PD], F32, kind="ExternalInput"),
    ]
    g_d = [
        nc.dram_tensor("g1", [PD], F32, kind="ExternalInput"),
        nc.dram_tensor("g2", [PD], F32, kind="ExternalInput"),
        nc.dram_tensor("g3", [PD], F32, kind="ExternalInput"),
    ]
    bln_d = [
        nc.dram_tensor("b1", [PD], F32, kind="ExternalInput"),
        nc.dram_tensor("b2", [PD], F32, kind="ExternalInput"),
        nc.dram_tensor("b3", [PD], F32, kind="ExternalInput"),
    ]
    p4w_d = nc.dram_tensor("p4_w", [PD, H], F32R, kind="ExternalInput")
    p4b_d = nc.dram_tensor("p4_b", [H], F32, kind="ExternalInput")
    biases_d = nc.dram_tensor("biases", [L, 2], F32R, kind="ExternalInput")
    ident_d = nc.dram_tensor("ident", [128, 128], F32R, kind="ExternalInput")
    sigperm_d = nc.dram_tensor("sigperm", [128, 128], F32R, kind="ExternalInput")
    ones_d = nc.dram_tensor("ones", [128, 128], F32R, kind="ExternalInput")
    y_d = nc.dram_tensor("y", [T, C], F32, kind="ExternalOutput")
    # per-core scratch holding exp(pos^T) rows, head-major [H*961]
    posdram = nc.dram_tensor("posdram", [H * L], F32)

    with tile.TileContext(nc) as tc:
        nc._allow_low_precision_reason = "float32r rounding of matmul operands is intended"
        with (
            tc.tile_pool(name="const", bufs=1) as constp,
            tc.tile_pool(name="mlp", bufs=1) as mlpp,
            tc.tile_pool(name="rpb", bufs=1) as rpbp,
            tc.tile_pool(name="win", bufs=1) as winp,   # per-window pools use explicit tags+bufs below
            tc.tile_pool(name="ps_tr", bufs=1, space="PSUM") as ps_tr,
            tc.tile_pool(name="ps_qk", bufs=3, space="PSUM") as ps_qk,
            tc.tile_pool(name="ps_big", bufs=2, space="PSUM") as ps_big,
            tc.tile_pool(name="ps_o", bufs=2, space="PSUM") as ps_o,
        ):
            # ---------------- Phase A: constants ----------------
            ident = constp.tile([128, 128], F32R)
            nc.sync.dma_start(ident[:], ident_d[:])
            sigperm = constp.tile([128, 128], F32R)
            nc.sync.dma_start(sigperm[:], sigperm_d[:])
            ones = constp.tile([128, 128], F32R)
            nc.sync.dma_start(ones[:], ones_d[:])

            qw = []
            for k in range(4):
                t = constp.tile([128, 3 * C], F32R, tag=f"qw{k}")
                nc.sync.dma_start(t[:], qkvw_d[k * 128:(k + 1) * 128, :])
                qw.append(t)
            # proj_w stored as eight base-0 [64, 512] tiles so each head's
            # O^T tile (always at partitions 0-63) can serve as lhsT directly
            pw64 = []
            for k in range(8):
                t = constp.tile([64, C], F32R, tag=f"pw{k}")
                nc.sync.dma_start(t[:], projw_d[k * 64:(k + 1) * 64, :])
                pw64.append(t)

            # q/k bias columns: qbT[p, j] = qkv_b[j*128 + p], j in 0..7
            qbT = constp.tile([128, 8], F32)
            nc.sync.dma_start(
                qbT[:], bass.AP(tensor=qkvb_d[:].tensor, offset=0, ap=[[1, 128], [128, 8]])
            )
            # v bias broadcast [1,512] -> [128,512]
            vb1 = constp.tile([1, C], F32)
            nc.sync.dma_start(vb1[:], qkvb_d[2 * C:3 * C].unsqueeze(0))
            vb_bc = constp.tile([128, C], F32)
            nc.gpsimd.partition_broadcast(vb_bc[:], vb1[:])
            # proj bias broadcast
            pb1 = constp.tile([1, C], F32)
            nc.sync.dma_start(pb1[:], projb_d[:].unsqueeze(0))
            pb_bc = constp.tile([128, C], F32)
            nc.gpsimd.partition_broadcast(pb_bc[:], pb1[:])
            eps_ap = constp.tile([PD, 1], F32)
            nc.gpsimd.memset(eps_ap[:], LN_EPS)

            # small MLP params
            pw_sb, pb_sb, g_sb, bln_sb = [], [], [], []
            for i in range(3):
                wt = mlpp.tile(list(pw_d[i].shape), F32R, tag=f"pw_sb{i}")
                nc.sync.dma_start(wt[:], pw_d[i][:])
                pw_sb.append(wt)
                bt = mlpp.tile([PD, 1], F32, tag=f"pb_sb{i}")
                nc.sync.dma_start(bt[:], pb_d[i][:].unsqueeze(1))
                pb_sb.append(bt)
                gt = mlpp.tile([PD, 1], F32, tag=f"g_sb{i}")
                nc.sync.dma_start(gt[:], g_d[i][:].unsqueeze(1))
                g_sb.append(gt)
                lt = mlpp.tile([PD, 1], F32, tag=f"bln_sb{i}")
                nc.sync.dma_start(lt[:], bln_d[i][:].unsqueeze(1))
                bln_sb.append(lt)
            p4w_sb = mlpp.tile([PD, H], F32R)
            nc.sync.dma_start(p4w_sb[:], p4w_d[:])
            p4b_sb = mlpp.tile([H, 1], F32)
            nc.sync.dma_start(p4b_sb[:], p4b_d[:].unsqueeze(1))

            # biases -> biasesT [2, 961] via PE transposes of [128,2] tiles
            biasesT = mlpp.tile([2, L], F32R)
            for i in range(8):
                rows = min(128, L - i * 128)
                rpad = rows + (rows % 2)
                bt = mlpp.tile([128, 2], F32R, tag="btile")
                if rpad != rows:
                    nc.gpsimd.memset(bt[:].bitcast(F32), 0.0)
                nc.sync.dma_start(bt[0:rows, :], biases_d[i * 128:i * 128 + rows, :])
                tp = ps_tr.tile([2, 128], F32, tag="trp")
                nc.tensor.transpose(tp[:, 0:rpad].bitcast(F32R), bt[0:rpad, :], ident[0:rpad, 0:rpad])
                nc.scalar.copy(biasesT[:, i * 128:i * 128 + rows], tp[:, 0:rows])

            # ---------------- Phase B: pos MLP (feature-on-partition) ----------
            segs = [(0, 512), (L - 512, 512)]  # overlap keeps fp32r free-size even
            h_cur = biasesT
            ln_scale = 1.0 / PD
            for li in range(3):
                kdim = 2 if li == 0 else PD
                z = mlpp.tile([PD, L], F32R, tag="z", bufs=2)
                xm = mlpp.tile([PD, L], F32R, tag="xm", bufs=2)
                sq = mlpp.tile([PD, L], F32R, tag="sq", bufs=2)
                mean = mlpp.tile([1, L], F32R, tag="mean", bufs=2)
                sd = mlpp.tile([1, L], F32, tag="sd", bufs=2)
                rstd = mlpp.tile([1, L], F32R, tag="rstd", bufs=2)
                hn = mlpp.tile([PD, L], F32R, tag=f"h{li % 2}", bufs=1)
                for s0, sl in segs:
                    zp = ps_qk.tile([PD, 512], F32, tag="qk")
                    nc.tensor.matmul(zp[:, 0:sl], pw_sb[li][0:kdim, :], h_cur[0:kdim, s0:s0 + sl],
                                     start=True, stop=True)
                    nc.vector.tensor_scalar(z[:, s0:s0 + sl], zp[:, 0:sl], pb_sb[li][:], None, op0=ADD)
                    mp = ps_o.tile([1, 512], F32, tag="o")
                    nc.tensor.matmul(mp[0:1, 0:sl], ones[0:PD, 0:1], z[:, s0:s0 + sl].bitcast(F32R),
                                     start=True, stop=True)
                    nc.scalar.activation(mean[:, s0:s0 + sl], mp[0:1, 0:sl], COPY, scale=ln_scale)
                    mb = ps_tr.tile([PD, 512], F32, tag="trp")
                    nc.tensor.matmul(mb[:, 0:sl], ones[0:1, 0:PD], mean[:, s0:s0 + sl],
                                     start=True, stop=True)
                    nc.vector.tensor_tensor(xm[:, s0:s0 + sl], z[:, s0:s0 + sl], mb[:, 0:sl], op=SUB)
                    nc.vector.tensor_tensor(sq[:, s0:s0 + sl], xm[:, s0:s0 + sl], xm[:, s0:s0 + sl], op=MULT)
                    vp = ps_o.tile([1, 512], F32, tag="o")
                    nc.tensor.matmul(vp[0:1, 0:sl], ones[0:PD, 0:1], sq[:, s0:s0 + sl],
                                     start=True, stop=True)
                    nc.scalar.activation(sd[:, s0:s0 + sl], vp[0:1, 0:sl], SQRT,
                                         bias=eps_ap[0:1, :], scale=ln_scale)
                    nc.vector.reciprocal(rstd[:, s0:s0 + sl], sd[:, s0:s0 + sl])
                    rb = ps_tr.tile([PD, 512], F32, tag="trp")
                    nc.tensor.matmul(rb[:, 0:sl], ones[0:1, 0:PD], rstd[:, s0:s0 + sl],
                                     start=True, stop=True)
                    nc.vector.tensor_tensor(hn[:, s0:s0 + sl], xm[:, s0:s0 + sl], rb[:, 0:sl], op=MULT)
                    # gamma * h + beta, then relu
                    nc.vector.tensor_scalar(hn[:, s0:s0 + sl], hn[:, s0:s0 + sl],
                                            g_sb[li][:], bln_sb[li][:], op0=MULT, op1=ADD)
                    nc.vector.tensor_scalar(hn[:, s0:s0 + sl], hn[:, s0:s0 + sl], 0.0, None, op0=MAX)
                h_cur = hn

            exp_posT = mlpp.tile([H, L], F32)
            for s0, sl in segs:
                pp = ps_qk.tile([H, 512], F32, tag="qk")
                nc.tensor.matmul(pp[:, 0:sl], p4w_sb[:], h_cur[:, s0:s0 + sl], start=True, stop=True)
                nc.scalar.activation(exp_posT[:, s0:s0 + sl], pp[:, 0:sl], EXP, bias=p4b_sb[:])
            nc.sync.dma_start(
                bass.AP(tensor=posdram[:].tensor, offset=0, ap=[[L, H], [1, L]]), exp_posT[:]
            )

            # ------------- Phase C: exp_rpb^T tiles [128, 256] x (H x 2) ----------
            # sigma-ordered gather (the DMA-legal order), then a permutation
            # transpose + plain transpose per 128-column half to undo sigma.
            exp_rpbT = [[rpbp.tile([128, N], F32, tag=f"rpb{h}_{c}", name=f"rpb{h}_{c}") for c in range(2)] for h in range(H)]
            for h in range(H):
                for c in range(2):
                    sig = rpbp.tile([128, N], F32, tag="rpbsig")
                    for mhl in range(8):
                        mh = c * 8 + mhl
                        src = bass.AP(tensor=posdram[:].tensor,
                                      offset=h * L + (15 - mh) * 31,
                                      ap=[[1, 16], [31, 16], [1, 16]])
                        nc.sync.dma_start(
                            sig[mhl * 16:(mhl + 1) * 16, :].rearrange("p (a b) -> p a b", b=16), src
                        )
                    for half in range(2):
                        t1 = ps_tr.tile([128, 128], F32, tag="trp")
                        nc.tensor.matmul(t1[:], sig[:, half * 128:(half + 1) * 128],
                                         sigperm[:].bitcast(F32), is_transpose=True)
                        tmp = rpbp.tile([128, 128], F32, tag="rpbtmp")
                        nc.scalar.copy(tmp[:], t1[:])
                        t2 = ps_tr.tile([128, 128], F32, tag="trp")
                        nc.tensor.transpose(t2[:], tmp[:], ident[:].bitcast(F32))
                        nc.vector.tensor_copy(exp_rpbT[h][c][:, half * 128:(half + 1) * 128], t2[:])

            # ---------------- Phase D: window loop ----------------
            def window_body(w):
                xa = []
                for c in range(2):
                    t = winp.tile([128, C], F32R, tag=f"xa{c}", bufs=2)
                    nc.sync.dma_start(t[:], x_d[w * N + c * 128: w * N + (c + 1) * 128, :])
                    xa.append(t)
                xT = []
                for k in range(4):
                    t = winp.tile([128, N], F32R, tag=f"xT{k}", bufs=2)
                    for c in range(2):
                        tp = ps_tr.tile([128, 128], F32, tag="trp")
                        nc.tensor.transpose(tp[:].bitcast(F32R), xa[c][:, k * 128:(k + 1) * 128], ident[:])
                        nc.scalar.copy(t[:, c * 128:(c + 1) * 128], tp[:])
                    xT.append(t)
                # q^T / k^T channel-major tiles (mi 0..3 = q heads 0-7, 4..7 = k)
                qkT = []
                for mi in range(8):
                    ps = ps_qk.tile([128, N], F32, tag="qk")
                    for k in range(4):
                        nc.tensor.matmul(ps[:], qw[k][:, mi * 128:(mi + 1) * 128], xT[k][:],
                                         start=(k == 0), stop=(k == 3))
                    t = winp.tile([128, N], F32R, tag=f"qkT{mi}", bufs=2)
                    nc.vector.tensor_scalar(t[:], ps[:], qbT[:, mi:mi + 1], None, op0=ADD)
                    qkT.append(t)
                # v token-major with ones column per head: [128, 8*65]
                vaug = []
                for c in range(2):
                    ps = ps_big.tile([128, C], F32, tag="big")
                    for k in range(4):
                        nc.tensor.matmul(ps[:], xT[k][:, c * 128:(c + 1) * 128], qw[k][:, 2 * C:3 * C],
                                         start=(k == 0), stop=(k == 3))
                    t = winp.tile([128, H * (HD + 1)], F32R, tag=f"vaug{c}", bufs=2)
                    nc.vector.tensor_tensor(
                        t[:].rearrange("p (h q) -> p h q", q=HD + 1)[:, :, 0:HD],
                        ps[:].rearrange("p (h q) -> p h q", q=HD),
                        vb_bc[:].rearrange("p (h q) -> p h q", q=HD),
                        op=ADD,
                    )
                    nc.gpsimd.memset(t[:].rearrange("p (h q) -> p h q", q=HD + 1)[:, :, HD:HD + 1].bitcast(F32), 1.0)
                    vaug.append(t)
                # attention per head
                oT = [winp.tile([64, N], F32R, tag=f"oT{i}", bufs=2, name=f"oT{i}") for i in range(H)]
                for h in range(H):
                    hp, sub = h // 2, h % 2
                    bp = sub * 64
                    ops = ps_o.tile([HD + 1, N], F32, tag="o")
                    for mc in range(2):
                        sps = ps_qk.tile([128, N], F32, tag="qk")
                        nc.tensor.matmul(sps[:], qkT[4 + hp][bp:bp + 64, mc * 128:(mc + 1) * 128],
                                         qkT[hp][bp:bp + 64, :], start=True, stop=True,
                                         tile_position=(bp, 0))
                        praw = winp.tile([128, N], F32, tag="praw", bufs=2)
                        nc.scalar.activation(praw[:], sps[:], EXP, scale=SCALE)
                        pTr = winp.tile([128, N], F32R, tag=f"pTr{mc}", bufs=2)
                        nc.gpsimd.tensor_tensor(pTr[:], praw[:], exp_rpbT[h][mc][:], op=MULT)
                        nc.tensor.matmul(ops[:], vaug[mc][:, h * (HD + 1):(h + 1) * (HD + 1)], pTr[:],
                                         start=(mc == 0), stop=(mc == 1))
                    rcp = winp.tile([1, N], F32, tag="rcp", bufs=3)
                    nc.vector.reciprocal(rcp[:], ops[HD:HD + 1, :])
                    rcpb = winp.tile([HD, N], F32, tag="rcpb", bufs=2)
                    nc.gpsimd.partition_broadcast(rcpb[:], rcp[:])
                    nc.vector.tensor_tensor(oT[h][:], ops[0:HD, :], rcpb[:], op=MULT)
                # proj: contract over c in 8 chunks of 64 (one per head tile)
                for c in range(2):
                    ps = ps_big.tile([128, C], F32, tag="big")
                    for k in range(8):
                        nc.tensor.matmul(ps[:], oT[k][:, c * 128:(c + 1) * 128], pw64[k][:],
                                         start=(k == 0), stop=(k == 7))
                    yo = winp.tile([128, C], F32, tag=f"yo{c}", bufs=2)
                    nc.vector.tensor_tensor(yo[:], ps[:], pb_bc[:], op=ADD)
                    nc.sync.dma_start(y_d[w * N + c * 128: w * N + (c + 1) * 128, :], yo[:])

            if repeat == 1:
                for w in range(wpc):
                    window_body(w)
            else:
                def rbody(i):
                    for w in range(wpc):
                        window_body(w)
                with tc.For_i(0, repeat, 1) as _:
                    rbody(_)

    nc.compile()
    return nc


_PROG_CACHE = {}


def _get_prog(wpc, repeat=1):
    key = (wpc, repeat)
    if key not in _PROG_CACHE:
        _PROG_CACHE[key] = build_program(wpc, repeat)
    return _PROG_CACHE[key]


def make_in_maps(inputs, wpc=WPC):
    ident, sigperm, ones = _host_consts()
    x = np.ascontiguousarray(np.asarray(inputs["x"], dtype=np.float32))
    shared = {
        "qkv_w": np.asarray(inputs["qkv_w"], np.float32),
        "qkv_b": np.asarray(inputs["qkv_b"], np.float32),
        "proj_w": np.asarray(inputs["proj_w"], np.float32),
        "proj_b": np.asarray(inputs["proj_b"], np.float32),
        "p1_w": np.asarray(inputs["p1_w"], np.float32),
        "p2_w": np.asarray(inputs["p2_w"], np.float32),
        "p3_w": np.asarray(inputs["p3_w"], np.float32),
        "p1_b": np.asarray(inputs["p1_b"], np.float32),
        "p2_b": np.asarray(inputs["p2_b"], np.float32),
        "p3_b": np.asarray(inputs["p3_b"], np.float32),
        "g1": np.asarray(inputs["g1"], np.float32),
        "g2": np.asarray(inputs["g2"], np.float32),
        "g3": np.asarray(inputs["g3"], np.float32),
        "b1": np.asarray(inputs["b1"], np.float32),
        "b2": np.asarray(inputs["b2"], np.float32),
        "b3": np.asarray(inputs["b3"], np.float32),
        "p4_w": np.asarray(inputs["p4_w"], np.float32),
        "p4_b": np.asarray(inputs["p4_b"], np.float32),
        "biases": np.asarray(inputs["biases"], np.float32),
        "ident": ident,
        "sigperm": sigperm,
        "ones": ones,
    }
    in_maps = []
    for cidx in range(NCORES):
        m = dict(shared)
        m["x"] = x[cidx * wpc:(cidx + 1) * wpc].reshape(wpc * N, C)
        in_maps.append(m)
    return in_maps


def kernel(**inputs):
    nc = _get_prog(WPC, 1)
    in_maps = make_in_maps(inputs, WPC)
    res = run_bass_kernel_spmd(nc, in_maps, list(range(NCORES)))
    out = np.concatenate(
        [res.results[c]["y"].reshape(WPC, N, C) for c in range(NCORES)], axis=0
    )
    return out.astype(np.float32)


if __name__ == "__main__":
    rng = np.random.default_rng(0)
    demo = {
        "x": rng.standard_normal((B, N, C), dtype=np.float32),
    }
    print("use test.py for the full check")


# revision 15
# speedup vs baseline: 6.1501x; 6.1501x over previous
"""Trainium2 Bass kernel for windowed multi-head attention with a dynamic
position-bias MLP (CrossFormer-style), data-parallel over windows on 8 cores.

Math per window (N=256 tokens, C=512 dim, H=8 heads, hd=64):
    qkv = x @ qkv_w + qkv_b ; q scaled by hd**-0.5
    attn = softmax(q @ k^T + rpb) ; out = (attn @ v) @ proj_w + proj_b
where rpb[h] = pos[rel_idx] and pos = MLP(biases) is a tiny 4-layer MLP
(LayerNorm + ReLU) applied to the 961 relative-offset rows, shared by all
windows.

Layout strategy on each NeuronCore (32 windows/core):
  - x is PE-transposed to x^T (channels on partitions).
  - q^T/k^T are produced channel-major ([c, t]); v token-major ([t, c]).
  - S^T = k^T.T @ q^T computed per head with 2-head row-packing (K=64).
  - P^T = exp(0.125*S^T) * exp_rpb^T   (softmax without max-subtraction:
    logits are O(1) by construction, exp cannot overflow; rpb enters as a
    multiplicative exp table).
  - O^T = v_aug.T @ P^T with a ones-column in v_aug producing the softmax
    denominator as row 64; rows are divided by it during evacuation.
  - y = proj applied with O^T directly as the stationary operand.
  - exp_rpb^T is gathered once per launch from DRAM with a structured
    (block-Toeplitz) access pattern in a reversed-within-16 row order (the
    only DMA-legal order), then fixed up with permutation-transposes.

All matmuls run as float32r (full fp32 data; ~1e-3 matmul rounding, 4x
faster than fp32 mode on the PE).
"""

import os
import sys

if "axon" not in os.environ.get("JAX_PLATFORMS", ""):
    os.environ["JAX_PLATFORMS"] = "axon"

for _p in (
    "/root/.axon_site",
    "/root/.axon_site/_ro/trn_rl_repo",
    "/root/.axon_site/_ro/pypackages",
    "/opt/trn_rl_repo",
):
    if os.path.isdir(_p) and _p not in sys.path:
        sys.path.append(_p)

import numpy as np

import concourse.bass as bass
import concourse.bacc as bacc
import concourse.mybir as mybir
import concourse.tile as tile
from concourse.bass_utils import run_bass_kernel_spmd

F32 = mybir.dt.float32
F32R = mybir.dt.float32r
EXP = mybir.ActivationFunctionType.Exp
SQRT = mybir.ActivationFunctionType.Sqrt
COPY = mybir.ActivationFunctionType.Copy
ADD = mybir.AluOpType.add
SUB = mybir.AluOpType.subtract
MULT = mybir.AluOpType.mult
MAX = mybir.AluOpType.max

B, N, C = 256, 256, 512
H, HD = 8, 64
PD, L = 32, 961  # pos-MLP width, (2*16-1)**2 offset rows
GH = GW = 16
NCORES = 8
WPC = B // NCORES  # windows per core
SCALE = HD ** -0.5
LN_EPS = 1e-5


def _host_consts():
    ident = np.eye(128, dtype=np.float32)
    sigma = np.array([(i // 16) * 16 + (15 - i % 16) for i in range(128)])
    sigperm = np.zeros((128, 128), np.float32)
    for i in range(128):
        sigperm[i, sigma[i]] = 1.0
    ones = np.ones((128, 128), np.float32)
    return ident, sigperm, ones


def build_program(wpc=WPC, repeat=1, has_qkv_b=True, has_proj_b=True):
    """Build the SPMD Bass program for one core handling `wpc` windows.

    repeat>1 wraps the steady-state window loop in a hardware For loop for
    wall-clock timing (the computation is idempotent)."""
    nc = bacc.Bacc("TRN2", num_devices=NCORES)
    T = wpc * N  # tokens per core

    x_d = nc.dram_tensor("x", [T, C], F32R, kind="ExternalInput")
    qkvw_d = nc.dram_tensor("qkv_w", [C, 3 * C], F32R, kind="ExternalInput")
    qkvb_d = nc.dram_tensor("qkv_b", [3 * C], F32, kind="ExternalInput")
    projw_d = nc.dram_tensor("proj_w", [C, C], F32R, kind="ExternalInput")
    projb_d = nc.dram_tensor("proj_b", [C], F32, kind="ExternalInput")
    pw_d = [
        nc.dram_tensor("p1_w", [2, PD], F32R, kind="ExternalInput"),
        nc.dram_tensor("p2_w", [PD, PD], F32R, kind="ExternalInput"),
        nc.dram_tensor("p3_w", [PD, PD], F32R, kind="ExternalInput"),
    ]
    pb_d = [
        nc.dram_tensor("p1_b", [PD], F32, kind="ExternalInput"),
        nc.dram_tensor("p2_b", [PD], F32, kind="ExternalInput"),
        nc.dram_tensor("p3_b", [PD], F32, kind="ExternalInput"),
    ]
    g_d = [
        nc.dram_tensor("g1", [PD], F32, kind="ExternalInput"),
        nc.dram_tensor("g2", [PD], F32, kind="ExternalInput"),
        nc.dram_tensor("g3", [PD], F32, kind="ExternalInput"),
    ]
    bln_d = [
        nc.dram_tensor("b1", [PD], F32, kind="ExternalInput"),
        nc.dram_tensor("b2", [PD], F32, kind="ExternalInput"),
        nc.dram_tensor("b3", [PD], F32, kind="ExternalInput"),
    ]
    p4w_d = nc.dram_tensor("p4_w", [PD, H], F32R, kind="ExternalInput")
    p4b_d = nc.dram_tensor("p4_b", [H], F32, kind="ExternalInput")
    biases_d = nc.dram_tensor("biases", [L, 2], F32R, kind="ExternalInput")
    ident_d = nc.dram_tensor("ident", [128, 128], F32R, kind="ExternalInput")
    sigperm_d = nc.dram_tensor("sigperm", [128, 128], F32R, kind="ExternalInput")
    ones_d = nc.dram_tensor("ones", [128, 128], F32R, kind="ExternalInput")
    y_d = nc.dram_tensor("y", [T, C], F32, kind="ExternalOutput")
    # per-core scratch holding exp(pos^T) rows, head-major [H*961]
    posdram = nc.dram_tensor("posdram", [H * L], F32)

    with tile.TileContext(nc) as tc:
        nc._allow_low_precision_reason = "float32r rounding of matmul operands is intended"
        with (
            tc.tile_pool(name="const", bufs=1) as constp,
            tc.tile_pool(name="mlp", bufs=1) as mlpp,
            tc.tile_pool(name="rpb", bufs=1) as rpbp,
            tc.tile_pool(name="win", bufs=1) as winp,   # per-window pools use explicit tags+bufs below
            tc.tile_pool(name="ps_tr", bufs=2, space="PSUM") as ps_tr,
            tc.tile_pool(name="psA", bufs=5, space="PSUM") as psA,
            tc.tile_pool(name="ps_o", bufs=1, space="PSUM") as ps_o,
        ):
            # ---------------- Phase A: constants ----------------
            ident = constp.tile([128, 128], F32R)
            nc.sync.dma_start(ident[:], ident_d[:])
            sigperm = constp.tile([128, 128], F32R)
            nc.sync.dma_start(sigperm[:], sigperm_d[:])
            ones = constp.tile([128, 128], F32R)
            nc.sync.dma_start(ones[:], ones_d[:])

            qw = []
            for k in range(4):
                t = constp.tile([128, 3 * C], F32R, tag=f"qw{k}")
                nc.sync.dma_start(t[:], qkvw_d[k * 128:(k + 1) * 128, :])
                qw.append(t)
            # proj_w stored as eight base-0 [64, 512] tiles so each head's
            # O^T tile (always at partitions 0-63) can serve as lhsT directly
            pw64 = []
            for k in range(8):
                t = constp.tile([64, C], F32R, tag=f"pw{k}")
                nc.sync.dma_start(t[:], projw_d[k * 64:(k + 1) * 64, :])
                pw64.append(t)

            # q/k bias columns: qbT[p, j] = qkv_b[j*128 + p], j in 0..7
            qbT = constp.tile([128, 8], F32)
            nc.sync.dma_start(
                qbT[:], bass.AP(tensor=qkvb_d[:].tensor, offset=0, ap=[[1, 128], [128, 8]])
            )
            # v bias broadcast [1,512] -> [128,512]
            vb1 = constp.tile([1, C], F32)
            nc.sync.dma_start(vb1[:], qkvb_d[2 * C:3 * C].unsqueeze(0))
            vb_bc = constp.tile([128, C], F32)
            nc.gpsimd.partition_broadcast(vb_bc[:], vb1[:])
            # proj bias broadcast
            pb1 = constp.tile([1, C], F32)
            nc.sync.dma_start(pb1[:], projb_d[:].unsqueeze(0))
            pb_bc = constp.tile([128, C], F32)
            nc.gpsimd.partition_broadcast(pb_bc[:], pb1[:])
            eps_ap = constp.tile([PD, 1], F32)
            nc.gpsimd.memset(eps_ap[:], LN_EPS)

            # small MLP params
            pw_sb, pb_sb, g_sb, bln_sb = [], [], [], []
            for i in range(3):
                wt = mlpp.tile(list(pw_d[i].shape), F32R, tag=f"pw_sb{i}")
                nc.sync.dma_start(wt[:], pw_d[i][:])
                pw_sb.append(wt)
                bt = mlpp.tile([PD, 1], F32, tag=f"pb_sb{i}")
                nc.sync.dma_start(bt[:], pb_d[i][:].unsqueeze(1))
                pb_sb.append(bt)
                gt = mlpp.tile([PD, 1], F32, tag=f"g_sb{i}")
                nc.sync.dma_start(gt[:], g_d[i][:].unsqueeze(1))
                g_sb.append(gt)
                lt = mlpp.tile([PD, 1], F32, tag=f"bln_sb{i}")
                nc.sync.dma_start(lt[:], bln_d[i][:].unsqueeze(1))
                bln_sb.append(lt)
            p4w_sb = mlpp.tile([PD, H], F32R)
            nc.sync.dma_start(p4w_sb[:], p4w_d[:])
            p4b_sb = mlpp.tile([H, 1], F32)
            nc.sync.dma_start(p4b_sb[:], p4b_d[:].unsqueeze(1))

            # biases -> biasesT [2, 961] via PE transposes of [128,2] tiles
            biasesT = mlpp.tile([2, L], F32R)
            for i in range(8):
                rows = min(128, L - i * 128)
                rpad = rows + (rows % 2)
                bt = mlpp.tile([128, 2], F32R, tag="btile")
                if rpad != rows:
                    nc.gpsimd.memset(bt[:].bitcast(F32), 0.0)
                nc.sync.dma_start(bt[0:rows, :], biases_d[i * 128:i * 128 + rows, :])
                tp = ps_tr.tile([2, 128], F32, tag="trp")
                nc.tensor.transpose(tp[:, 0:rpad].bitcast(F32R), bt[0:rpad, :], ident[0:rpad, 0:rpad])
                nc.scalar.copy(biasesT[:, i * 128:i * 128 + rows], tp[:, 0:rows])

            # ---------------- Phase B: pos MLP (feature-on-partition) ----------
            segs = [(0, 512), (L - 512, 512)]  # overlap keeps fp32r free-size even
            h_cur = biasesT
            ln_scale = 1.0 / PD
            for li in range(3):
                kdim = 2 if li == 0 else PD
                z = mlpp.tile([PD, L], F32R, tag="z", bufs=2)
                xm = mlpp.tile([PD, L], F32R, tag="xm", bufs=2)
                sq = mlpp.tile([PD, L], F32R, tag="sq", bufs=2)
                mean = mlpp.tile([1, L], F32R, tag="mean", bufs=2)
                sd = mlpp.tile([1, L], F32, tag="sd", bufs=2)
                rstd = mlpp.tile([1, L], F32R, tag="rstd", bufs=2)
                hn = mlpp.tile([PD, L], F32R, tag=f"h{li % 2}", bufs=1)
                for s0, sl in segs:
                    zp = psA.tile([PD, 512], F32, tag="A")
                    nc.tensor.matmul(zp[:, 0:sl], pw_sb[li][0:kdim, :], h_cur[0:kdim, s0:s0 + sl],
                                     start=True, stop=True)
                    nc.vector.tensor_scalar(z[:, s0:s0 + sl], zp[:, 0:sl], pb_sb[li][:], None, op0=ADD)
                    mp = ps_o.tile([1, 512], F32, tag="o")
                    nc.tensor.matmul(mp[0:1, 0:sl], ones[0:PD, 0:1], z[:, s0:s0 + sl].bitcast(F32R),
                                     start=True, stop=True)
                    nc.scalar.activation(mean[:, s0:s0 + sl], mp[0:1, 0:sl], COPY, scale=ln_scale)
                    mb = ps_tr.tile([PD, 512], F32, tag="trp")
                    nc.tensor.matmul(mb[:, 0:sl], ones[0:1, 0:PD], mean[:, s0:s0 + sl],
                                     start=True, stop=True)
                    nc.vector.tensor_tensor(xm[:, s0:s0 + sl], z[:, s0:s0 + sl], mb[:, 0:sl], op=SUB)
                    nc.vector.tensor_tensor(sq[:, s0:s0 + sl], xm[:, s0:s0 + sl], xm[:, s0:s0 + sl], op=MULT)
                    vp = ps_o.tile([1, 512], F32, tag="o")
                    nc.tensor.matmul(vp[0:1, 0:sl], ones[0:PD, 0:1], sq[:, s0:s0 + sl],
                                     start=True, stop=True)
                    nc.scalar.activation(sd[:, s0:s0 + sl], vp[0:1, 0:sl], SQRT,
                                         bias=eps_ap[0:1, :], scale=ln_scale)
                    nc.vector.reciprocal(rstd[:, s0:s0 + sl], sd[:, s0:s0 + sl])
                    rb = ps_tr.tile([PD, 512], F32, tag="trp")
                    nc.tensor.matmul(rb[:, 0:sl], ones[0:1, 0:PD], rstd[:, s0:s0 + sl],
                                     start=True, stop=True)
                    nc.vector.tensor_tensor(hn[:, s0:s0 + sl], xm[:, s0:s0 + sl], rb[:, 0:sl], op=MULT)
                    # gamma * h + beta, then relu
                    nc.vector.tensor_scalar(hn[:, s0:s0 + sl], hn[:, s0:s0 + sl],
                                            g_sb[li][:], bln_sb[li][:], op0=MULT, op1=ADD)
                    nc.vector.tensor_scalar(hn[:, s0:s0 + sl], hn[:, s0:s0 + sl], 0.0, None, op0=MAX)
                h_cur = hn

            exp_posT = mlpp.tile([H, L], F32)
            for s0, sl in segs:
                pp = psA.tile([H, 512], F32, tag="A")
                nc.tensor.matmul(pp[:, 0:sl], p4w_sb[:], h_cur[:, s0:s0 + sl], start=True, stop=True)
                nc.scalar.activation(exp_posT[:, s0:s0 + sl], pp[:, 0:sl], EXP, bias=p4b_sb[:])
            nc.sync.dma_start(
                bass.AP(tensor=posdram[:].tensor, offset=0, ap=[[L, H], [1, L]]), exp_posT[:]
            )

            # ------------- Phase C: exp_rpb^T tiles [128, 512] per head ----------
            # sigma-ordered gather (the DMA-legal order), then a permutation
            # transpose + plain transpose per 128-column half to undo sigma.
            # Tile h holds both m-chunks side by side: cols [mc*256, mc*256+256).
            exp_rpbT = [rpbp.tile([128, 2 * N], F32, tag=f"rpb{h}", name=f"rpb{h}") for h in range(H)]
            for h in range(H):
                for c in range(2):
                    sig = rpbp.tile([128, N], F32, tag="rpbsig")
                    for mhl in range(8):
                        mh = c * 8 + mhl
                        src = bass.AP(tensor=posdram[:].tensor,
                                      offset=h * L + (15 - mh) * 31,
                                      ap=[[1, 16], [31, 16], [1, 16]])
                        nc.sync.dma_start(
                            sig[mhl * 16:(mhl + 1) * 16, :].rearrange("p (a b) -> p a b", b=16), src
                        )
                    for half in range(2):
                        t1 = ps_tr.tile([128, 128], F32, tag="trp")
                        nc.tensor.matmul(t1[:], sig[:, half * 128:(half + 1) * 128],
                                         sigperm[:].bitcast(F32), is_transpose=True)
                        tmp = rpbp.tile([128, 128], F32, tag="rpbtmp")
                        nc.scalar.copy(tmp[:], t1[:])
                        t2 = ps_tr.tile([128, 128], F32, tag="trp")
                        nc.tensor.transpose(t2[:], tmp[:], ident[:].bitcast(F32))
                        nc.vector.tensor_copy(
                            exp_rpbT[h][:, c * N + half * 128: c * N + (half + 1) * 128], t2[:])

            # ---------------- Phase D: window loop ----------------
            def window_body(w):
                xa = []
                for c in range(2):
                    t = winp.tile([128, C], F32R, tag=f"xa{c}", bufs=2)
                    nc.sync.dma_start(t[:], x_d[w * N + c * 128: w * N + (c + 1) * 128, :])
                    xa.append(t)
                xT = []
                for k in range(4):
                    t = winp.tile([128, N], F32R, tag=f"xT{k}", bufs=2)
                    tp = ps_tr.tile([128, N], F32, tag="trp")
                    for c in range(2):
                        nc.tensor.transpose(tp[:, c * 128:(c + 1) * 128].bitcast(F32R),
                                            xa[c][:, k * 128:(k + 1) * 128], ident[:])
                    nc.scalar.copy(t[:], tp[:])
                    xT.append(t)
                # q^T / k^T channel-major tiles (mi 0..3 = q heads 0-7, 4..7 = k),
                # paired into [128, 512] psum tiles for single-op evacuation
                qkT = []
                for mp in range(4):
                    ps = psA.tile([128, 2 * N], F32, tag="A")
                    for half in range(2):
                        mi = mp * 2 + half
                        for k in range(4):
                            nc.tensor.matmul(ps[:, half * N:(half + 1) * N],
                                             qw[k][:, mi * 128:(mi + 1) * 128], xT[k][:],
                                             start=(k == 0), stop=(k == 3))
                    t = winp.tile([128, 2 * N], F32R, tag=f"qkT{mp}", bufs=2)
                    if has_qkv_b:
                        for half in range(2):
                            mi = mp * 2 + half
                            nc.vector.tensor_scalar(t[:, half * N:(half + 1) * N],
                                                    ps[:, half * N:(half + 1) * N],
                                                    qbT[:, mi:mi + 1], None, op0=ADD)
                    else:
                        nc.vector.tensor_copy(t[:], ps[:])
                    qkT.append(t)

                # v token-major with ones column per head: [128, 8*65]
                vaug = []
                for c in range(2):
                    ps = psA.tile([128, C], F32, tag="A")
                    for k in range(4):
                        nc.tensor.matmul(ps[:], xT[k][:, c * 128:(c + 1) * 128], qw[k][:, 2 * C:3 * C],
                                         start=(k == 0), stop=(k == 3))
                    t = winp.tile([128, H * (HD + 1)], F32R, tag=f"vaug{c}", bufs=2)
                    if has_qkv_b:
                        nc.vector.tensor_tensor(
                            t[:].rearrange("p (h q) -> p h q", q=HD + 1)[:, :, 0:HD],
                            ps[:].rearrange("p (h q) -> p h q", q=HD),
                            vb_bc[:].rearrange("p (h q) -> p h q", q=HD),
                            op=ADD,
                        )
                    else:
                        nc.vector.tensor_copy(
                            t[:].rearrange("p (h q) -> p h q", q=HD + 1)[:, :, 0:HD],
                            ps[:].rearrange("p (h q) -> p h q", q=HD),
                        )
                    nc.gpsimd.memset(t[:].rearrange("p (h q) -> p h q", q=HD + 1)[:, :, HD:HD + 1].bitcast(F32), 1.0)
                    vaug.append(t)
                # attention per head; both m-chunks of S^T live in one [128,512] psum
                oT = [winp.tile([64, N], F32R, tag=f"oT{i}", bufs=2, name=f"oT{i}") for i in range(H)]
                for h in range(H):
                    hp, sub = h // 2, h % 2
                    bp = sub * 64
                    kq = qkT[2 + hp // 2]
                    qq = qkT[hp // 2]
                    kbase = (hp % 2) * N
                    qbase = (hp % 2) * N
                    sps = psA.tile([128, 2 * N], F32, tag="A")
                    for mc in range(2):
                        nc.tensor.matmul(sps[:, mc * N:(mc + 1) * N],
                                         kq[bp:bp + 64, kbase + mc * 128: kbase + (mc + 1) * 128],
                                         qq[bp:bp + 64, qbase:qbase + N], start=True, stop=True,
                                         tile_position=(bp, 0))
                    praw = winp.tile([128, 2 * N], F32, tag="praw", bufs=2)
                    nc.scalar.activation(praw[:], sps[:], EXP, scale=SCALE)
                    pTr = winp.tile([128, 2 * N], F32R, tag="pTr", bufs=2)
                    nc.vector.tensor_tensor(pTr[:], praw[:], exp_rpbT[h][:], op=MULT)
                    ops = ps_o.tile([HD + 1, N], F32, tag="o")
                    for mc in range(2):
                        nc.tensor.matmul(ops[:], vaug[mc][:, h * (HD + 1):(h + 1) * (HD + 1)],
                                         pTr[:, mc * N:(mc + 1) * N],
                                         start=(mc == 0), stop=(mc == 1))
                    rcp = winp.tile([1, N], F32, tag="rcp", bufs=3)
                    nc.vector.reciprocal(rcp[:], ops[HD:HD + 1, :])
                    rcpb = winp.tile([HD, N], F32, tag="rcpb", bufs=2)
                    nc.gpsimd.partition_broadcast(rcpb[:], rcp[:])
                    nc.vector.tensor_tensor(oT[h][:], ops[0:HD, :], rcpb[:], op=MULT)
                # proj: contract over c in 8 chunks of 64 (one per head tile)
                for c in range(2):
                    ps = psA.tile([128, C], F32, tag="A")
                    for k in range(8):
                        nc.tensor.matmul(ps[:], oT[k][:, c * 128:(c + 1) * 128], pw64[k][:],
                                         start=(k == 0), stop=(k == 7))
                    yo = winp.tile([128, C], F32, tag=f"yo{c}", bufs=2)
                    if has_proj_b:
                        nc.vector.tensor_tensor(yo[:], ps[:], pb_bc[:], op=ADD)
                    else:
                        nc.vector.tensor_copy(yo[:], ps[:])
                    nc.sync.dma_start(y_d[w * N + c * 128: w * N + (c + 1) * 128, :], yo[:])

            if repeat == 1:
                for w in range(wpc):
                    window_body(w)
            else:
                def rbody(i):
                    for w in range(wpc):
                        window_body(w)
                with tc.For_i(0, repeat, 1) as _:
                    rbody(_)

    nc.compile()
    return nc


_PROG_CACHE = {}


def _get_prog(wpc, repeat=1, has_qkv_b=True, has_proj_b=True):
    key = (wpc, repeat, has_qkv_b, has_proj_b)
    if key not in _PROG_CACHE:
        _PROG_CACHE[key] = build_program(wpc, repeat, has_qkv_b, has_proj_b)
    return _PROG_CACHE[key]


def make_in_maps(inputs, wpc=WPC):
    ident, sigperm, ones = _host_consts()
    x = np.ascontiguousarray(np.asarray(inputs["x"], dtype=np.float32))
    shared = {
        "qkv_w": np.asarray(inputs["qkv_w"], np.float32),
        "qkv_b": np.asarray(inputs["qkv_b"], np.float32),
        "proj_w": np.asarray(inputs["proj_w"], np.float32),
        "proj_b": np.asarray(inputs["proj_b"], np.float32),
        "p1_w": np.asarray(inputs["p1_w"], np.float32),
        "p2_w": np.asarray(inputs["p2_w"], np.float32),
        "p3_w": np.asarray(inputs["p3_w"], np.float32),
        "p1_b": np.asarray(inputs["p1_b"], np.float32),
        "p2_b": np.asarray(inputs["p2_b"], np.float32),
        "p3_b": np.asarray(inputs["p3_b"], np.float32),
        "g1": np.asarray(inputs["g1"], np.float32),
        "g2": np.asarray(inputs["g2"], np.float32),
        "g3": np.asarray(inputs["g3"], np.float32),
        "b1": np.asarray(inputs["b1"], np.float32),
        "b2": np.asarray(inputs["b2"], np.float32),
        "b3": np.asarray(inputs["b3"], np.float32),
        "p4_w": np.asarray(inputs["p4_w"], np.float32),
        "p4_b": np.asarray(inputs["p4_b"], np.float32),
        "biases": np.asarray(inputs["biases"], np.float32),
        "ident": ident,
        "sigperm": sigperm,
        "ones": ones,
    }
    in_maps = []
    for cidx in range(NCORES):
        m = dict(shared)
        m["x"] = x[cidx * wpc:(cidx + 1) * wpc].reshape(wpc * N, C)
        in_maps.append(m)
    return in_maps


def kernel(**inputs):
    nc = _get_prog(WPC, 1)
    in_maps = make_in_maps(inputs, WPC)
    res = run_bass_kernel_spmd(nc, in_maps, list(range(NCORES)))
    out = np.concatenate(
        [res.results[c]["y"].reshape(WPC, N, C) for c in range(NCORES)], axis=0
    )
    return out.astype(np.float32)


if __name__ == "__main__":
    rng = np.random.default_rng(0)
    demo = {
        "x": rng.standard_normal((B, N, C), dtype=np.float32),
    }
    print("use test.py for the full check")


# revision 26
# speedup vs baseline: 6.7974x; 1.1053x over previous
"""Trainium2 Bass kernel for windowed multi-head attention with a dynamic
position-bias MLP (CrossFormer-style), data-parallel over windows on 8 cores.

Math per window (N=256 tokens, C=512 dim, H=8 heads, hd=64):
    qkv = x @ qkv_w + qkv_b ; q scaled by hd**-0.5
    attn = softmax(q @ k^T + rpb) ; out = (attn @ v) @ proj_w + proj_b
where rpb[h] = pos[rel_idx] and pos = MLP(biases) is a tiny 4-layer MLP
(LayerNorm + ReLU) applied to the 961 relative-offset rows, shared by all
windows.

Layout strategy on each NeuronCore (32 windows/core):
  - x is PE-transposed to x^T (channels on partitions).
  - q^T/k^T are produced channel-major ([c, t]); v token-major ([t, c]).
  - S^T = k^T.T @ q^T computed per head with 2-head row-packing (K=64).
  - P^T = exp(0.125*S^T) * exp_rpb^T   (softmax without max-subtraction:
    logits are O(1) by construction, exp cannot overflow; rpb enters as a
    multiplicative exp table).
  - O^T = v_aug.T @ P^T with a ones-column in v_aug producing the softmax
    denominator as row 64; rows are divided by it during evacuation.
  - y = proj applied with O^T directly as the stationary operand.
  - exp_rpb^T is gathered once per launch from DRAM with a structured
    (block-Toeplitz) access pattern in a reversed-within-16 row order (the
    only DMA-legal order), then fixed up with permutation-transposes.

All matmuls run as float32r (full fp32 data; ~1e-3 matmul rounding, 4x
faster than fp32 mode on the PE).
"""

import os
import sys

if "axon" not in os.environ.get("JAX_PLATFORMS", ""):
    os.environ["JAX_PLATFORMS"] = "axon"

for _p in (
    "/root/.axon_site",
    "/root/.axon_site/_ro/trn_rl_repo",
    "/root/.axon_site/_ro/pypackages",
    "/opt/trn_rl_repo",
):
    if os.path.isdir(_p) and _p not in sys.path:
        sys.path.append(_p)

import numpy as np

import concourse.bass as bass
import concourse.bacc as bacc
import concourse.mybir as mybir
import concourse.tile as tile
from concourse.bass_utils import run_bass_kernel_spmd

F32 = mybir.dt.float32
F32R = mybir.dt.float32r
EXP = mybir.ActivationFunctionType.Exp
SQRT = mybir.ActivationFunctionType.Sqrt
COPY = mybir.ActivationFunctionType.Copy
ADD = mybir.AluOpType.add
SUB = mybir.AluOpType.subtract
MULT = mybir.AluOpType.mult
MAX = mybir.AluOpType.max

B, N, C = 256, 256, 512
H, HD = 8, 64
PD, L = 32, 961  # pos-MLP width, (2*16-1)**2 offset rows
GH = GW = 16
NCORES = 8
WPC = B // NCORES  # windows per core
SCALE = HD ** -0.5
LN_EPS = 1e-5


def _host_consts():
    ident = np.eye(128, dtype=np.float32)
    sigma = np.array([(i // 16) * 16 + (15 - i % 16) for i in range(128)])
    sigperm = np.zeros((128, 128), np.float32)
    for i in range(128):
        sigperm[i, sigma[i]] = 1.0
    ones = np.ones((128, 128), np.float32)
    return ident, sigperm, ones


def build_program(wpc=WPC, repeat=1, has_qkv_b=True, has_proj_b=True):
    """Build the SPMD Bass program for one core handling `wpc` windows.

    repeat>1 wraps the steady-state window loop in a hardware For loop for
    wall-clock timing (the computation is idempotent)."""
    nc = bacc.Bacc("TRN2", num_devices=NCORES)
    T = wpc * N  # tokens per core

    x_d = nc.dram_tensor("x", [T, C], F32R, kind="ExternalInput")
    qkvw_d = nc.dram_tensor("qkv_w", [C, 3 * C], F32R, kind="ExternalInput")
    qkvb_d = nc.dram_tensor("qkv_b", [3 * C], F32, kind="ExternalInput")
    projw_d = nc.dram_tensor("proj_w", [C, C], F32R, kind="ExternalInput")
    projb_d = nc.dram_tensor("proj_b", [C], F32, kind="ExternalInput")
    pw_d = [
        nc.dram_tensor("p1_w", [2, PD], F32R, kind="ExternalInput"),
        nc.dram_tensor("p2_w", [PD, PD], F32R, kind="ExternalInput"),
        nc.dram_tensor("p3_w", [PD, PD], F32R, kind="ExternalInput"),
    ]
    pb_d = [
        nc.dram_tensor("p1_b", [PD], F32, kind="ExternalInput"),
        nc.dram_tensor("p2_b", [PD], F32, kind="ExternalInput"),
        nc.dram_tensor("p3_b", [PD], F32, kind="ExternalInput"),
    ]
    g_d = [
        nc.dram_tensor("g1", [PD], F32, kind="ExternalInput"),
        nc.dram_tensor("g2", [PD], F32, kind="ExternalInput"),
        nc.dram_tensor("g3", [PD], F32, kind="ExternalInput"),
    ]
    bln_d = [
        nc.dram_tensor("b1", [PD], F32, kind="ExternalInput"),
        nc.dram_tensor("b2", [PD], F32, kind="ExternalInput"),
        nc.dram_tensor("b3", [PD], F32, kind="ExternalInput"),
    ]
    p4w_d = nc.dram_tensor("p4_w", [PD, H], F32R, kind="ExternalInput")
    p4b_d = nc.dram_tensor("p4_b", [H], F32, kind="ExternalInput")
    biases_d = nc.dram_tensor("biases", [L, 2], F32R, kind="ExternalInput")
    ident_d = nc.dram_tensor("ident", [128, 128], F32R, kind="ExternalInput")
    sigperm_d = nc.dram_tensor("sigperm", [128, 128], F32R, kind="ExternalInput")
    ones_d = nc.dram_tensor("ones", [128, 128], F32R, kind="ExternalInput")
    y_d = nc.dram_tensor("y", [T, C], F32, kind="ExternalOutput")
    # per-core scratch holding exp(pos^T) rows, head-major [H*961]
    posdram = nc.dram_tensor("posdram", [H * L], F32)

    with tile.TileContext(nc) as tc:
        nc._allow_low_precision_reason = "float32r rounding of matmul operands is intended"
        with (
            tc.tile_pool(name="const", bufs=1) as constp,
            tc.tile_pool(name="mlp", bufs=1) as mlpp,
            tc.tile_pool(name="rpb", bufs=1) as rpbp,
            tc.tile_pool(name="win", bufs=1) as winp,   # per-window pools use explicit tags+bufs below
            tc.tile_pool(name="ps_tr", bufs=1, space="PSUM") as ps_tr,
            tc.tile_pool(name="psA", bufs=5, space="PSUM") as psA,
            tc.tile_pool(name="ps_dn", bufs=2, space="PSUM") as ps_dn,
        ):
            # ---------------- Phase A: constants ----------------
            ident = constp.tile([128, 128], F32R)
            nc.scalar.dma_start(ident[:], ident_d[:])
            sigperm = constp.tile([128, 128], F32R)
            nc.scalar.dma_start(sigperm[:], sigperm_d[:])
            ones = constp.tile([128, 128], F32R)
            nc.scalar.dma_start(ones[:], ones_d[:])

            qw = []
            for k in range(4):
                t = constp.tile([128, 3 * C], F32R, tag=f"qw{k}")
                nc.scalar.dma_start(t[:], qkvw_d[k * 128:(k + 1) * 128, :])
                qw.append(t)
            pw64 = []
            for k in range(8):
                t = constp.tile([64, C], F32R, tag=f"pw{k}")
                nc.scalar.dma_start(t[:], projw_d[k * 64:(k + 1) * 64, :])
                pw64.append(t)

            # q/k bias columns: qbT[p, j] = qkv_b[j*128 + p], j in 0..7
            qbT = constp.tile([128, 8], F32)
            nc.sync.dma_start(
                qbT[:], bass.AP(tensor=qkvb_d[:].tensor, offset=0, ap=[[1, 128], [128, 8]])
            )
            # v bias broadcast [1,512] -> [128,512]
            vb1 = constp.tile([1, C], F32)
            nc.sync.dma_start(vb1[:], qkvb_d[2 * C:3 * C].unsqueeze(0))
            vb_bc = constp.tile([128, C], F32)
            nc.gpsimd.partition_broadcast(vb_bc[:], vb1[:])
            # proj bias broadcast
            pb1 = constp.tile([1, C], F32)
            nc.sync.dma_start(pb1[:], projb_d[:].unsqueeze(0))
            pb_bc = constp.tile([128, C], F32)
            nc.gpsimd.partition_broadcast(pb_bc[:], pb1[:])
            eps_ap = constp.tile([PD, 1], F32)
            nc.gpsimd.memset(eps_ap[:], LN_EPS)

            # small MLP params
            pw_sb, pb_sb, g_sb, bln_sb = [], [], [], []
            for i in range(3):
                wt = mlpp.tile(list(pw_d[i].shape), F32R, tag=f"pw_sb{i}")
                nc.sync.dma_start(wt[:], pw_d[i][:])
                pw_sb.append(wt)
                bt = mlpp.tile([PD, 1], F32, tag=f"pb_sb{i}")
                nc.sync.dma_start(bt[:], pb_d[i][:].unsqueeze(1))
                pb_sb.append(bt)
                gt = mlpp.tile([PD, 1], F32, tag=f"g_sb{i}")
                nc.sync.dma_start(gt[:], g_d[i][:].unsqueeze(1))
                g_sb.append(gt)
                lt = mlpp.tile([PD, 1], F32, tag=f"bln_sb{i}")
                nc.sync.dma_start(lt[:], bln_d[i][:].unsqueeze(1))
                bln_sb.append(lt)
            p4w_sb = mlpp.tile([PD, H], F32R)
            nc.sync.dma_start(p4w_sb[:], p4w_d[:])
            p4b_sb = mlpp.tile([H, 1], F32)
            nc.sync.dma_start(p4b_sb[:], p4b_d[:].unsqueeze(1))

            # biases -> biasesT [2, 961] via PE transposes of [128,2] tiles
            biasesT = mlpp.tile([2, L], F32R)
            for i in range(8):
                rows = min(128, L - i * 128)
                rpad = rows + (rows % 2)
                bt = mlpp.tile([128, 2], F32R, tag="btile")
                if rpad != rows:
                    nc.gpsimd.memset(bt[:].bitcast(F32), 0.0)
                nc.sync.dma_start(bt[0:rows, :], biases_d[i * 128:i * 128 + rows, :])
                tp = ps_tr.tile([2, 128], F32, tag="trp")
                nc.tensor.transpose(tp[:, 0:rpad].bitcast(F32R), bt[0:rpad, :], ident[0:rpad, 0:rpad])
                nc.scalar.copy(biasesT[:, i * 128:i * 128 + rows], tp[:, 0:rows])

            # ---------------- Phase B: pos MLP (feature-on-partition) ----------
            segs = [(0, 512), (L - 512, 512)]  # overlap keeps fp32r free-size even
            h_cur = biasesT
            ln_scale = 1.0 / PD
            for li in range(3):
                kdim = 2 if li == 0 else PD
                z = mlpp.tile([PD, L], F32R, tag="z", bufs=1)
                xm = mlpp.tile([PD, L], F32R, tag="xm", bufs=1)
                sq = mlpp.tile([PD, L], F32R, tag="sq", bufs=1)
                mean = mlpp.tile([1, L], F32R, tag="mean", bufs=2)
                sd = mlpp.tile([1, L], F32, tag="sd", bufs=2)
                rstd = mlpp.tile([1, L], F32R, tag="rstd", bufs=2)
                hn = mlpp.tile([PD, L], F32R, tag=f"h{li % 2}", bufs=1)
                for s0, sl in segs:
                    zp = psA.tile([PD, 512], F32, tag="A")
                    nc.tensor.matmul(zp[:, 0:sl], pw_sb[li][0:kdim, :], h_cur[0:kdim, s0:s0 + sl],
                                     start=True, stop=True)
                    nc.vector.tensor_scalar(z[:, s0:s0 + sl], zp[:, 0:sl], pb_sb[li][:], None, op0=ADD)
                    mp = ps_dn.tile([1, 512], F32, tag="dn")
                    nc.tensor.matmul(mp[0:1, 0:sl], ones[0:PD, 0:1], z[:, s0:s0 + sl].bitcast(F32R),
                                     start=True, stop=True)
                    nc.scalar.activation(mean[:, s0:s0 + sl], mp[0:1, 0:sl], COPY, scale=ln_scale)
                    mb = ps_tr.tile([PD, 512], F32, tag="trp")
                    nc.tensor.matmul(mb[:, 0:sl], ones[0:1, 0:PD], mean[:, s0:s0 + sl],
                                     start=True, stop=True)
                    nc.vector.tensor_tensor(xm[:, s0:s0 + sl], z[:, s0:s0 + sl], mb[:, 0:sl], op=SUB)
                    nc.vector.tensor_tensor(sq[:, s0:s0 + sl], xm[:, s0:s0 + sl], xm[:, s0:s0 + sl], op=MULT)
                    vp = ps_dn.tile([1, 512], F32, tag="dn")
                    nc.tensor.matmul(vp[0:1, 0:sl], ones[0:PD, 0:1], sq[:, s0:s0 + sl],
                                     start=True, stop=True)
                    nc.scalar.activation(sd[:, s0:s0 + sl], vp[0:1, 0:sl], SQRT,
                                         bias=eps_ap[0:1, :], scale=ln_scale)
                    nc.vector.reciprocal(rstd[:, s0:s0 + sl], sd[:, s0:s0 + sl])
                    rb = ps_tr.tile([PD, 512], F32, tag="trp")
                    nc.tensor.matmul(rb[:, 0:sl], ones[0:1, 0:PD], rstd[:, s0:s0 + sl],
                                     start=True, stop=True)
                    nc.vector.tensor_tensor(hn[:, s0:s0 + sl], xm[:, s0:s0 + sl], rb[:, 0:sl], op=MULT)
                    # gamma * h + beta, then relu
                    nc.vector.tensor_scalar(hn[:, s0:s0 + sl], hn[:, s0:s0 + sl],
                                            g_sb[li][:], bln_sb[li][:], op0=MULT, op1=ADD)
                    nc.vector.tensor_scalar(hn[:, s0:s0 + sl], hn[:, s0:s0 + sl], 0.0, None, op0=MAX)
                h_cur = hn

            posT = mlpp.tile([H, L], F32)
            for s0, sl in segs:
                pp = psA.tile([H, 512], F32, tag="A")
                nc.tensor.matmul(pp[:, 0:sl], p4w_sb[:], h_cur[:, s0:s0 + sl], start=True, stop=True)
                # pre-scale by 1/SCALE: the window-loop exp applies scale to S+rpb
                nc.vector.tensor_scalar(posT[:, s0:s0 + sl], pp[:, 0:sl], p4b_sb[:], 1.0 / SCALE,
                                        op0=ADD, op1=MULT)
            nc.sync.dma_start(
                bass.AP(tensor=posdram[:].tensor, offset=0, ap=[[L, H], [1, L]]), posT[:]
            )

            # ------------- Phase C: exp_rpb^T tiles [128, 512] per head ----------
            # sigma-ordered gather (the DMA-legal order), then a permutation
            # transpose + plain transpose per 128-column half to undo sigma.
            # Tile h holds both m-chunks side by side: cols [mc*256, mc*256+256).
            rpbT = [rpbp.tile([128, 2 * N], F32R, tag=f"rpb{h}", name=f"rpb{h}") for h in range(H)]
            for h in range(H):
                for c in range(2):
                    sig = rpbp.tile([128, N], F32, tag="rpbsig")
                    for mhl in range(8):
                        mh = c * 8 + mhl
                        src = bass.AP(tensor=posdram[:].tensor,
                                      offset=h * L + (15 - mh) * 31,
                                      ap=[[1, 16], [31, 16], [1, 16]])
                        nc.scalar.dma_start(
                            sig[mhl * 16:(mhl + 1) * 16, :].rearrange("p (a b) -> p a b", b=16), src
                        )
                    for half in range(2):
                        t1 = ps_tr.tile([128, 128], F32, tag="trp")
                        nc.tensor.matmul(t1[:], sig[:, half * 128:(half + 1) * 128],
                                         sigperm[:].bitcast(F32), is_transpose=True)
                        tmp = rpbp.tile([128, 128], F32, tag="rpbtmp")
                        nc.scalar.copy(tmp[:], t1[:])
                        t2 = ps_tr.tile([128, 128], F32, tag="trp")
                        nc.tensor.transpose(t2[:], tmp[:], ident[:].bitcast(F32))
                        nc.vector.tensor_copy(
                            rpbT[h][:, c * N + half * 128: c * N + (half + 1) * 128], t2[:])

            # ---------------- Phase D: window loop ----------------
            def window_body(w):
                xa = []
                for c in range(2):
                    t = winp.tile([128, C], F32R, tag=f"xa{c}", bufs=2)
                    nc.sync.dma_start(t[:], x_d[w * N + c * 128: w * N + (c + 1) * 128, :])
                    xa.append(t)
                xT = []
                for k in range(4):
                    t = winp.tile([128, N], F32R, tag=f"xT{k}", bufs=3)
                    tp = ps_tr.tile([128, N], F32, tag="trp")
                    for c in range(2):
                        nc.tensor.transpose(tp[:, c * 128:(c + 1) * 128].bitcast(F32R),
                                            xa[c][:, k * 128:(k + 1) * 128], ident[:])
                    nc.scalar.copy(t[:], tp[:])
                    xT.append(t)
                # q^T / k^T channel-major tiles (mi 0..3 = q heads 0-7, 4..7 = k),
                # paired into [128, 512] psum tiles for single-op evacuation
                qkT = []
                for mp in range(4):
                    ps = psA.tile([128, 2 * N], F32, tag="A")
                    for half in range(2):
                        mi = mp * 2 + half
                        for k in range(4):
                            nc.tensor.matmul(ps[:, half * N:(half + 1) * N],
                                             qw[k][:, mi * 128:(mi + 1) * 128], xT[k][:],
                                             start=(k == 0), stop=(k == 3))
                    t = winp.tile([128, 2 * N], F32R, tag=f"qkT{mp}", bufs=2)
                    if has_qkv_b:
                        for half in range(2):
                            mi = mp * 2 + half
                            nc.vector.tensor_scalar(t[:, half * N:(half + 1) * N],
                                                    ps[:, half * N:(half + 1) * N],
                                                    qbT[:, mi:mi + 1], None, op0=ADD)
                    else:
                        nc.scalar.copy(t[:], ps[:])
                    qkT.append(t)

                # v token-major, plain [128, 512] tiles
                vv = []
                for c in range(2):
                    ps = psA.tile([128, C], F32, tag="A")
                    for k in range(4):
                        nc.tensor.matmul(ps[:], xT[k][:, c * 128:(c + 1) * 128], qw[k][:, 2 * C:3 * C],
                                         start=(k == 0), stop=(k == 3))
                    t = winp.tile([128, C], F32R, tag=f"vv{c}", bufs=2)
                    if has_qkv_b:
                        nc.vector.tensor_tensor(t[:], ps[:], vb_bc[:], op=ADD)
                    else:
                        nc.scalar.copy(t[:], ps[:])
                    vv.append(t)
                # attention: S^T per head with rpb^T accumulated in PSUM via an
                # identity matmul, then a single exp evacuation to f32r P^T.
                pT = [None] * H
                spss = [None] * H
                for hp in range(4):
                    for sub in range(2):
                        h = hp * 2 + sub
                        bp = sub * 64
                        kq = qkT[2 + hp // 2]
                        qq = qkT[hp // 2]
                        base = (hp % 2) * N
                        sps = psA.tile([128, 2 * N], F32, tag="A", name=f"sps{h}")
                        for mc in range(2):
                            nc.tensor.matmul(sps[:, mc * N:(mc + 1) * N],
                                             kq[bp:bp + 64, base + mc * 128: base + (mc + 1) * 128],
                                             qq[bp:bp + 64, base:base + N],
                                             start=(mc == 0), stop=False,
                                             tile_position=(bp, 0))
                        spss[h] = sps
                    for sub in range(2):
                        h = hp * 2 + sub
                        # += rpb^T (identity matmul accumulation closes the group)
                        nc.tensor.matmul(spss[h][:], ident[:], rpbT[h][:], start=False, stop=True)
                        t = winp.tile([128, 2 * N], F32R, tag="pT", bufs=4, name=f"pT{h}")
                        nc.scalar.activation(t[:], spss[h][:], EXP, scale=SCALE)
                        pT[h] = t
                # PV per head (base-0), denominators via accumulating ones-matmuls
                oT = [winp.tile([64, N], F32R, tag=f"oT{i}", bufs=2, name=f"oT{i}") for i in range(H)]
                for j in range(4):
                    h0, h1 = 2 * j, 2 * j + 1
                    dn = ps_dn.tile([1, 2 * N], F32, tag="dn")
                    for sub, h in ((0, h0), (1, h1)):
                        ops = psA.tile([64, N], F32, tag="A", name=f"ops{h}")
                        for mc in range(2):
                            nc.tensor.matmul(ops[:], vv[mc][:, h * HD:(h + 1) * HD],
                                             pT[h][:, mc * N:(mc + 1) * N],
                                             start=(mc == 0), stop=(mc == 1))
                        for mc in range(2):
                            nc.tensor.matmul(dn[0:1, sub * N:(sub + 1) * N],
                                             ones[0:128, 0:1],
                                             pT[h][:, mc * N:(mc + 1) * N],
                                             start=(mc == 0), stop=(mc == 1))
                        if sub == 0:
                            ops0 = ops
                    rcp = winp.tile([1, 2 * N], F32, tag="rcp", bufs=4)
                    nc.vector.reciprocal(rcp[:], dn[:])
                    for sub, h, op_t in ((0, h0, ops0), (1, h1, ops)):
                        rcb = winp.tile([HD, N], F32, tag="rcb", bufs=3)
                        nc.gpsimd.partition_broadcast(rcb[:], rcp[0:1, sub * N:(sub + 1) * N])
                        nc.vector.tensor_tensor(oT[h][:], op_t[:], rcb[:], op=MULT)
                # proj: contract over c in 8 chunks of 64 (one per head tile)
                for c in range(2):
                    ps = psA.tile([128, C], F32, tag="A")
                    for k in range(8):
                        nc.tensor.matmul(ps[:], oT[k][:, c * 128:(c + 1) * 128], pw64[k][:],
                                         start=(k == 0), stop=(k == 7))
                    yo = winp.tile([128, C], F32, tag=f"yo{c}", bufs=2)
                    if has_proj_b:
                        nc.vector.tensor_tensor(yo[:], ps[:], pb_bc[:], op=ADD)
                    else:
                        nc.scalar.copy(yo[:], ps[:])
                    nc.sync.dma_start(y_d[w * N + c * 128: w * N + (c + 1) * 128, :], yo[:])

            if repeat == 1:
                for w in range(wpc):
                    window_body(w)
            else:
                def rbody(i):
                    for w in range(wpc):
                        window_body(w)
                with tc.For_i(0, repeat, 1) as _:
                    rbody(_)

    nc.compile()
    return nc


_PROG_CACHE = {}


def _get_prog(wpc, repeat=1, has_qkv_b=True, has_proj_b=True):
    key = (wpc, repeat, has_qkv_b, has_proj_b)
    if key not in _PROG_CACHE:
        _PROG_CACHE[key] = build_program(wpc, repeat, has_qkv_b, has_proj_b)
    return _PROG_CACHE[key]


def make_in_maps(inputs, wpc=WPC):
    ident, sigperm, ones = _host_consts()
    x = np.ascontiguousarray(np.asarray(inputs["x"], dtype=np.float32))
    shared = {
        "qkv_w": np.asarray(inputs["qkv_w"], np.float32),
        "qkv_b": np.asarray(inputs["qkv_b"], np.float32),
        "proj_w": np.asarray(inputs["proj_w"], np.float32),
        "proj_b": np.asarray(inputs["proj_b"], np.float32),
        "p1_w": np.asarray(inputs["p1_w"], np.float32),
        "p2_w": np.asarray(inputs["p2_w"], np.float32),
        "p3_w": np.asarray(inputs["p3_w"], np.float32),
        "p1_b": np.asarray(inputs["p1_b"], np.float32),
        "p2_b": np.asarray(inputs["p2_b"], np.float32),
        "p3_b": np.asarray(inputs["p3_b"], np.float32),
        "g1": np.asarray(inputs["g1"], np.float32),
        "g2": np.asarray(inputs["g2"], np.float32),
        "g3": np.asarray(inputs["g3"], np.float32),
        "b1": np.asarray(inputs["b1"], np.float32),
        "b2": np.asarray(inputs["b2"], np.float32),
        "b3": np.asarray(inputs["b3"], np.float32),
        "p4_w": np.asarray(inputs["p4_w"], np.float32),
        "p4_b": np.asarray(inputs["p4_b"], np.float32),
        "biases": np.asarray(inputs["biases"], np.float32),
        "ident": ident,
        "sigperm": sigperm,
        "ones": ones,
    }
    in_maps = []
    for cidx in range(NCORES):
        m = dict(shared)
        m["x"] = x[cidx * wpc:(cidx + 1) * wpc].reshape(wpc * N, C)
        in_maps.append(m)
    return in_maps


def kernel(**inputs):
    nc = _get_prog(WPC, 1)
    in_maps = make_in_maps(inputs, WPC)
    res = run_bass_kernel_spmd(nc, in_maps, list(range(NCORES)))
    out = np.concatenate(
        [res.results[c]["y"].reshape(WPC, N, C) for c in range(NCORES)], axis=0
    )
    return out.astype(np.float32)


if __name__ == "__main__":
    rng = np.random.default_rng(0)
    demo = {
        "x": rng.standard_normal((B, N, C), dtype=np.float32),
    }
    print("use test.py for the full check")


# revision 27
# speedup vs baseline: 6.8885x; 1.0134x over previous
"""Trainium2 Bass kernel for windowed multi-head attention with a dynamic
position-bias MLP (CrossFormer-style), data-parallel over windows on 8 cores.

Math per window (N=256 tokens, C=512 dim, H=8 heads, hd=64):
    qkv = x @ qkv_w + qkv_b ; q scaled by hd**-0.5
    attn = softmax(q @ k^T + rpb) ; out = (attn @ v) @ proj_w + proj_b
where rpb[h] = pos[rel_idx] and pos = MLP(biases) is a tiny 4-layer MLP
(LayerNorm + ReLU) applied to the 961 relative-offset rows, shared by all
windows.

Layout strategy on each NeuronCore (32 windows/core):
  - x is PE-transposed to x^T (channels on partitions).
  - q^T/k^T are produced channel-major ([c, t]); v token-major ([t, c]).
  - S^T = k^T.T @ q^T computed per head with 2-head row-packing (K=64).
  - P^T = exp(0.125*S^T) * exp_rpb^T   (softmax without max-subtraction:
    logits are O(1) by construction, exp cannot overflow; rpb enters as a
    multiplicative exp table).
  - O^T = v_aug.T @ P^T with a ones-column in v_aug producing the softmax
    denominator as row 64; rows are divided by it during evacuation.
  - y = proj applied with O^T directly as the stationary operand.
  - exp_rpb^T is gathered once per launch from DRAM with a structured
    (block-Toeplitz) access pattern in a reversed-within-16 row order (the
    only DMA-legal order), then fixed up with permutation-transposes.

All matmuls run as float32r (full fp32 data; ~1e-3 matmul rounding, 4x
faster than fp32 mode on the PE).
"""

import os
import sys

if "axon" not in os.environ.get("JAX_PLATFORMS", ""):
    os.environ["JAX_PLATFORMS"] = "axon"

for _p in (
    "/root/.axon_site",
    "/root/.axon_site/_ro/trn_rl_repo",
    "/root/.axon_site/_ro/pypackages",
    "/opt/trn_rl_repo",
):
    if os.path.isdir(_p) and _p not in sys.path:
        sys.path.append(_p)

import numpy as np

import concourse.bass as bass
import concourse.bacc as bacc
import concourse.mybir as mybir
import concourse.tile as tile
from concourse.bass_utils import run_bass_kernel_spmd

F32 = mybir.dt.float32
F32R = mybir.dt.float32r
BF16 = mybir.dt.bfloat16
EXP = mybir.ActivationFunctionType.Exp
SQRT = mybir.ActivationFunctionType.Sqrt
COPY = mybir.ActivationFunctionType.Copy
ADD = mybir.AluOpType.add
SUB = mybir.AluOpType.subtract
MULT = mybir.AluOpType.mult
MAX = mybir.AluOpType.max

B, N, C = 256, 256, 512
H, HD = 8, 64
PD, L = 32, 961  # pos-MLP width, (2*16-1)**2 offset rows
GH = GW = 16
NCORES = 8
WPC = B // NCORES  # windows per core
SCALE = HD ** -0.5
LN_EPS = 1e-5


def _host_consts():
    ident = np.eye(128, dtype=np.float32)
    sigma = np.array([(i // 16) * 16 + (15 - i % 16) for i in range(128)])
    sigperm = np.zeros((128, 128), np.float32)
    for i in range(128):
        sigperm[i, sigma[i]] = 1.0
    ones = np.ones((128, 128), np.float32)
    return ident, sigperm, ones


def build_program(wpc=WPC, repeat=1, has_qkv_b=True, has_proj_b=True):
    """Build the SPMD Bass program for one core handling `wpc` windows.

    repeat>1 wraps the steady-state window loop in a hardware For loop for
    wall-clock timing (the computation is idempotent)."""
    nc = bacc.Bacc("TRN2", num_devices=NCORES)
    T = wpc * N  # tokens per core

    x_d = nc.dram_tensor("x", [T, C], F32R, kind="ExternalInput")
    qkvw_d = nc.dram_tensor("qkv_w", [C, 3 * C], F32R, kind="ExternalInput")
    qkvb_d = nc.dram_tensor("qkv_b", [3 * C], F32, kind="ExternalInput")
    projw_d = nc.dram_tensor("proj_w", [C, C], F32R, kind="ExternalInput")
    projb_d = nc.dram_tensor("proj_b", [C], F32, kind="ExternalInput")
    pw_d = [
        nc.dram_tensor("p1_w", [2, PD], F32R, kind="ExternalInput"),
        nc.dram_tensor("p2_w", [PD, PD], F32R, kind="ExternalInput"),
        nc.dram_tensor("p3_w", [PD, PD], F32R, kind="ExternalInput"),
    ]
    pb_d = [
        nc.dram_tensor("p1_b", [PD], F32, kind="ExternalInput"),
        nc.dram_tensor("p2_b", [PD], F32, kind="ExternalInput"),
        nc.dram_tensor("p3_b", [PD], F32, kind="ExternalInput"),
    ]
    g_d = [
        nc.dram_tensor("g1", [PD], F32, kind="ExternalInput"),
        nc.dram_tensor("g2", [PD], F32, kind="ExternalInput"),
        nc.dram_tensor("g3", [PD], F32, kind="ExternalInput"),
    ]
    bln_d = [
        nc.dram_tensor("b1", [PD], F32, kind="ExternalInput"),
        nc.dram_tensor("b2", [PD], F32, kind="ExternalInput"),
        nc.dram_tensor("b3", [PD], F32, kind="ExternalInput"),
    ]
    p4w_d = nc.dram_tensor("p4_w", [PD, H], F32R, kind="ExternalInput")
    p4b_d = nc.dram_tensor("p4_b", [H], F32, kind="ExternalInput")
    biases_d = nc.dram_tensor("biases", [L, 2], F32R, kind="ExternalInput")
    ident_d = nc.dram_tensor("ident", [128, 128], F32R, kind="ExternalInput")
    sigperm_d = nc.dram_tensor("sigperm", [128, 128], F32R, kind="ExternalInput")
    ones_d = nc.dram_tensor("ones", [128, 128], F32R, kind="ExternalInput")
    y_d = nc.dram_tensor("y", [T, C], F32, kind="ExternalOutput")
    # per-core scratch holding exp(pos^T) rows, head-major [H*961]
    posdram = nc.dram_tensor("posdram", [H * L], F32)

    with tile.TileContext(nc) as tc:
        nc._allow_low_precision_reason = "float32r rounding of matmul operands is intended"
        with (
            tc.tile_pool(name="const", bufs=1) as constp,
            tc.tile_pool(name="mlp", bufs=1) as mlpp,
            tc.tile_pool(name="rpb", bufs=1) as rpbp,
            tc.tile_pool(name="win", bufs=1) as winp,   # per-window pools use explicit tags+bufs below
            tc.tile_pool(name="ps_tr", bufs=1, space="PSUM") as ps_tr,
            tc.tile_pool(name="psA", bufs=5, space="PSUM") as psA,
            tc.tile_pool(name="ps_dn", bufs=2, space="PSUM") as ps_dn,
        ):
            # ---------------- Phase A: constants ----------------
            ident = constp.tile([128, 128], F32R)
            nc.scalar.dma_start(ident[:], ident_d[:])
            identb = constp.tile([128, 128], BF16)
            nc.scalar.activation(identb[:], ident[:].bitcast(F32), COPY)
            sigperm = constp.tile([128, 128], F32R)
            nc.scalar.dma_start(sigperm[:], sigperm_d[:])
            ones = constp.tile([128, 128], F32R)
            nc.scalar.dma_start(ones[:], ones_d[:])

            qw = []
            for k in range(4):
                t = constp.tile([128, 3 * C], F32R, tag=f"qw{k}")
                nc.scalar.dma_start(t[:], qkvw_d[k * 128:(k + 1) * 128, :])
                qw.append(t)
            pw64 = []
            for k in range(8):
                t = constp.tile([64, C], F32R, tag=f"pw{k}")
                nc.scalar.dma_start(t[:], projw_d[k * 64:(k + 1) * 64, :])
                pw64.append(t)

            # q/k bias columns: qbT[p, j] = qkv_b[j*128 + p], j in 0..7
            qbT = constp.tile([128, 8], F32)
            nc.sync.dma_start(
                qbT[:], bass.AP(tensor=qkvb_d[:].tensor, offset=0, ap=[[1, 128], [128, 8]])
            )
            # v bias broadcast [1,512] -> [128,512]
            vb1 = constp.tile([1, C], F32)
            nc.sync.dma_start(vb1[:], qkvb_d[2 * C:3 * C].unsqueeze(0))
            vb_bc = constp.tile([128, C], F32)
            nc.gpsimd.partition_broadcast(vb_bc[:], vb1[:])
            # proj bias broadcast
            pb1 = constp.tile([1, C], F32)
            nc.sync.dma_start(pb1[:], projb_d[:].unsqueeze(0))
            pb_bc = constp.tile([128, C], F32)
            nc.gpsimd.partition_broadcast(pb_bc[:], pb1[:])
            eps_ap = constp.tile([PD, 1], F32)
            nc.gpsimd.memset(eps_ap[:], LN_EPS)

            # small MLP params
            pw_sb, pb_sb, g_sb, bln_sb = [], [], [], []
            for i in range(3):
                wt = mlpp.tile(list(pw_d[i].shape), F32R, tag=f"pw_sb{i}")
                nc.sync.dma_start(wt[:], pw_d[i][:])
                pw_sb.append(wt)
                bt = mlpp.tile([PD, 1], F32, tag=f"pb_sb{i}")
                nc.sync.dma_start(bt[:], pb_d[i][:].unsqueeze(1))
                pb_sb.append(bt)
                gt = mlpp.tile([PD, 1], F32, tag=f"g_sb{i}")
                nc.sync.dma_start(gt[:], g_d[i][:].unsqueeze(1))
                g_sb.append(gt)
                lt = mlpp.tile([PD, 1], F32, tag=f"bln_sb{i}")
                nc.sync.dma_start(lt[:], bln_d[i][:].unsqueeze(1))
                bln_sb.append(lt)
            p4w_sb = mlpp.tile([PD, H], F32R)
            nc.sync.dma_start(p4w_sb[:], p4w_d[:])
            p4b_sb = mlpp.tile([H, 1], F32)
            nc.sync.dma_start(p4b_sb[:], p4b_d[:].unsqueeze(1))

            # biases -> biasesT [2, 961] via PE transposes of [128,2] tiles
            biasesT = mlpp.tile([2, L], F32R)
            for i in range(8):
                rows = min(128, L - i * 128)
                rpad = rows + (rows % 2)
                bt = mlpp.tile([128, 2], F32R, tag="btile")
                if rpad != rows:
                    nc.gpsimd.memset(bt[:].bitcast(F32), 0.0)
                nc.sync.dma_start(bt[0:rows, :], biases_d[i * 128:i * 128 + rows, :])
                tp = ps_tr.tile([2, 128], F32, tag="trp")
                nc.tensor.transpose(tp[:, 0:rpad].bitcast(F32R), bt[0:rpad, :], ident[0:rpad, 0:rpad])
                nc.scalar.copy(biasesT[:, i * 128:i * 128 + rows], tp[:, 0:rows])

            # ---------------- Phase B: pos MLP (feature-on-partition) ----------
            segs = [(0, 512), (L - 512, 512)]  # overlap keeps fp32r free-size even
            h_cur = biasesT
            ln_scale = 1.0 / PD
            for li in range(3):
                kdim = 2 if li == 0 else PD
                z = mlpp.tile([PD, L], F32R, tag="z", bufs=1)
                xm = mlpp.tile([PD, L], F32R, tag="xm", bufs=1)
                sq = mlpp.tile([PD, L], F32R, tag="sq", bufs=1)
                mean = mlpp.tile([1, L], F32R, tag="mean", bufs=2)
                sd = mlpp.tile([1, L], F32, tag="sd", bufs=2)
                rstd = mlpp.tile([1, L], F32R, tag="rstd", bufs=2)
                hn = mlpp.tile([PD, L], F32R, tag=f"h{li % 2}", bufs=1)
                for s0, sl in segs:
                    zp = psA.tile([PD, 512], F32, tag="A")
                    nc.tensor.matmul(zp[:, 0:sl], pw_sb[li][0:kdim, :], h_cur[0:kdim, s0:s0 + sl],
                                     start=True, stop=True)
                    nc.vector.tensor_scalar(z[:, s0:s0 + sl], zp[:, 0:sl], pb_sb[li][:], None, op0=ADD)
                    mp = ps_dn.tile([1, 512], F32, tag="dn")
                    nc.tensor.matmul(mp[0:1, 0:sl], ones[0:PD, 0:1], z[:, s0:s0 + sl].bitcast(F32R),
                                     start=True, stop=True)
                    nc.scalar.activation(mean[:, s0:s0 + sl], mp[0:1, 0:sl], COPY, scale=ln_scale)
                    mb = ps_tr.tile([PD, 512], F32, tag="trp")
                    nc.tensor.matmul(mb[:, 0:sl], ones[0:1, 0:PD], mean[:, s0:s0 + sl],
                                     start=True, stop=True)
                    nc.vector.tensor_tensor(xm[:, s0:s0 + sl], z[:, s0:s0 + sl], mb[:, 0:sl], op=SUB)
                    nc.vector.tensor_tensor(sq[:, s0:s0 + sl], xm[:, s0:s0 + sl], xm[:, s0:s0 + sl], op=MULT)
                    vp = ps_dn.tile([1, 512], F32, tag="dn")
                    nc.tensor.matmul(vp[0:1, 0:sl], ones[0:PD, 0:1], sq[:, s0:s0 + sl],
                                     start=True, stop=True)
                    nc.scalar.activation(sd[:, s0:s0 + sl], vp[0:1, 0:sl], SQRT,
                                         bias=eps_ap[0:1, :], scale=ln_scale)
                    nc.vector.reciprocal(rstd[:, s0:s0 + sl], sd[:, s0:s0 + sl])
                    rb = ps_tr.tile([PD, 512], F32, tag="trp")
                    nc.tensor.matmul(rb[:, 0:sl], ones[0:1, 0:PD], rstd[:, s0:s0 + sl],
                                     start=True, stop=True)
                    nc.vector.tensor_tensor(hn[:, s0:s0 + sl], xm[:, s0:s0 + sl], rb[:, 0:sl], op=MULT)
                    # gamma * h + beta, then relu
                    nc.vector.tensor_scalar(hn[:, s0:s0 + sl], hn[:, s0:s0 + sl],
                                            g_sb[li][:], bln_sb[li][:], op0=MULT, op1=ADD)
                    nc.vector.tensor_scalar(hn[:, s0:s0 + sl], hn[:, s0:s0 + sl], 0.0, None, op0=MAX)
                h_cur = hn

            posT = mlpp.tile([H, L], F32)
            for s0, sl in segs:
                pp = psA.tile([H, 512], F32, tag="A")
                nc.tensor.matmul(pp[:, 0:sl], p4w_sb[:], h_cur[:, s0:s0 + sl], start=True, stop=True)
                # pre-scale by 1/SCALE: the window-loop exp applies scale to S+rpb
                nc.vector.tensor_scalar(posT[:, s0:s0 + sl], pp[:, 0:sl], p4b_sb[:], 1.0 / SCALE,
                                        op0=ADD, op1=MULT)
            nc.sync.dma_start(
                bass.AP(tensor=posdram[:].tensor, offset=0, ap=[[L, H], [1, L]]), posT[:]
            )

            # ------------- Phase C: exp_rpb^T tiles [128, 512] per head ----------
            # sigma-ordered gather (the DMA-legal order), then a permutation
            # transpose + plain transpose per 128-column half to undo sigma.
            # Tile h holds both m-chunks side by side: cols [mc*256, mc*256+256).
            rpbT = [rpbp.tile([128, 2 * N], BF16, tag=f"rpb{h}", name=f"rpb{h}") for h in range(H)]
            for h in range(H):
                for c in range(2):
                    sig = rpbp.tile([128, N], F32, tag="rpbsig")
                    for mhl in range(8):
                        mh = c * 8 + mhl
                        src = bass.AP(tensor=posdram[:].tensor,
                                      offset=h * L + (15 - mh) * 31,
                                      ap=[[1, 16], [31, 16], [1, 16]])
                        nc.scalar.dma_start(
                            sig[mhl * 16:(mhl + 1) * 16, :].rearrange("p (a b) -> p a b", b=16), src
                        )
                    for half in range(2):
                        t1 = ps_tr.tile([128, 128], F32, tag="trp")
                        nc.tensor.matmul(t1[:], sig[:, half * 128:(half + 1) * 128],
                                         sigperm[:].bitcast(F32), is_transpose=True)
                        tmp = rpbp.tile([128, 128], F32, tag="rpbtmp")
                        nc.scalar.copy(tmp[:], t1[:])
                        t2 = ps_tr.tile([128, 128], F32, tag="trp")
                        nc.tensor.transpose(t2[:], tmp[:], ident[:].bitcast(F32))
                        nc.vector.tensor_copy(
                            rpbT[h][:, c * N + half * 128: c * N + (half + 1) * 128], t2[:])

            # ---------------- Phase D: window loop ----------------
            def window_body(w):
                xa = []
                for c in range(2):
                    t = winp.tile([128, C], F32R, tag=f"xa{c}", bufs=2)
                    nc.sync.dma_start(t[:], x_d[w * N + c * 128: w * N + (c + 1) * 128, :])
                    xa.append(t)
                xT = []
                for k in range(4):
                    t = winp.tile([128, N], F32R, tag=f"xT{k}", bufs=3)
                    tp = ps_tr.tile([128, N], F32, tag="trp")
                    for c in range(2):
                        nc.tensor.transpose(tp[:, c * 128:(c + 1) * 128].bitcast(F32R),
                                            xa[c][:, k * 128:(k + 1) * 128], ident[:])
                    nc.scalar.copy(t[:], tp[:])
                    xT.append(t)
                # q^T / k^T channel-major tiles (mi 0..3 = q heads 0-7, 4..7 = k),
                # paired into [128, 512] psum tiles for single-op evacuation
                qkT = []
                for mp in range(4):
                    ps = psA.tile([128, 2 * N], F32, tag="A")
                    for half in range(2):
                        mi = mp * 2 + half
                        for k in range(4):
                            nc.tensor.matmul(ps[:, half * N:(half + 1) * N],
                                             qw[k][:, mi * 128:(mi + 1) * 128], xT[k][:],
                                             start=(k == 0), stop=(k == 3))
                    t = winp.tile([128, 2 * N], F32R, tag=f"qkT{mp}", bufs=2)
                    if has_qkv_b:
                        for half in range(2):
                            mi = mp * 2 + half
                            nc.vector.tensor_scalar(t[:, half * N:(half + 1) * N],
                                                    ps[:, half * N:(half + 1) * N],
                                                    qbT[:, mi:mi + 1], None, op0=ADD)
                    else:
                        nc.scalar.copy(t[:], ps[:])
                    qkT.append(t)

                # v token-major, plain [128, 512] tiles
                vv = []
                for c in range(2):
                    ps = psA.tile([128, C], F32, tag="A")
                    for k in range(4):
                        nc.tensor.matmul(ps[:], xT[k][:, c * 128:(c + 1) * 128], qw[k][:, 2 * C:3 * C],
                                         start=(k == 0), stop=(k == 3))
                    t = winp.tile([128, C], F32R, tag=f"vv{c}", bufs=2)
                    if has_qkv_b:
                        nc.vector.tensor_tensor(t[:], ps[:], vb_bc[:], op=ADD)
                    else:
                        nc.scalar.copy(t[:], ps[:])
                    vv.append(t)
                # attention: S^T per head with rpb^T accumulated in PSUM via an
                # identity matmul, then a single exp evacuation to f32r P^T.
                pT = [None] * H
                spss = [None] * H
                for hp in range(4):
                    for sub in range(2):
                        h = hp * 2 + sub
                        bp = sub * 64
                        kq = qkT[2 + hp // 2]
                        qq = qkT[hp // 2]
                        base = (hp % 2) * N
                        sps = psA.tile([128, 2 * N], F32, tag="A", name=f"sps{h}")
                        for mc in range(2):
                            nc.tensor.matmul(sps[:, mc * N:(mc + 1) * N],
                                             kq[bp:bp + 64, base + mc * 128: base + (mc + 1) * 128],
                                             qq[bp:bp + 64, base:base + N],
                                             start=(mc == 0), stop=False,
                                             tile_position=(bp, 0))
                        spss[h] = sps
                    for sub in range(2):
                        h = hp * 2 + sub
                        # += rpb^T (identity matmul accumulation closes the group)
                        nc.tensor.matmul(spss[h][:], identb[:], rpbT[h][:], start=False, stop=True)
                        t = winp.tile([128, 2 * N], F32R, tag="pT", bufs=4, name=f"pT{h}")
                        nc.scalar.activation(t[:], spss[h][:], EXP, scale=SCALE)
                        pT[h] = t
                # PV per head (base-0), denominators via accumulating ones-matmuls
                oT = [winp.tile([64, N], F32R, tag=f"oT{i}", bufs=2, name=f"oT{i}") for i in range(H)]
                for j in range(4):
                    h0, h1 = 2 * j, 2 * j + 1
                    dn = ps_dn.tile([1, 2 * N], F32, tag="dn")
                    for sub, h in ((0, h0), (1, h1)):
                        ops = psA.tile([64, N], F32, tag="A", name=f"ops{h}")
                        for mc in range(2):
                            nc.tensor.matmul(ops[:], vv[mc][:, h * HD:(h + 1) * HD],
                                             pT[h][:, mc * N:(mc + 1) * N],
                                             start=(mc == 0), stop=(mc == 1))
                        for mc in range(2):
                            nc.tensor.matmul(dn[0:1, sub * N:(sub + 1) * N],
                                             ones[0:128, 0:1],
                                             pT[h][:, mc * N:(mc + 1) * N],
                                             start=(mc == 0), stop=(mc == 1))
                        if sub == 0:
                            ops0 = ops
                    rcp = winp.tile([1, 2 * N], F32, tag="rcp", bufs=4)
                    nc.vector.reciprocal(rcp[:], dn[:])
                    for sub, h, op_t in ((0, h0, ops0), (1, h1, ops)):
                        rcb = winp.tile([HD, N], F32, tag="rcb", bufs=3)
                        nc.gpsimd.partition_broadcast(rcb[:], rcp[0:1, sub * N:(sub + 1) * N])
                        nc.vector.tensor_tensor(oT[h][:], op_t[:], rcb[:], op=MULT)
                # proj: contract over c in 8 chunks of 64 (one per head tile)
                for c in range(2):
                    ps = psA.tile([128, C], F32, tag="A")
                    for k in range(8):
                        nc.tensor.matmul(ps[:], oT[k][:, c * 128:(c + 1) * 128], pw64[k][:],
                                         start=(k == 0), stop=(k == 7))
                    yo = winp.tile([128, C], F32, tag=f"yo{c}", bufs=2)
                    if has_proj_b:
                        nc.vector.tensor_tensor(yo[:], ps[:], pb_bc[:], op=ADD)
                    else:
                        nc.scalar.copy(yo[:], ps[:])
                    nc.sync.dma_start(y_d[w * N + c * 128: w * N + (c + 1) * 128, :], yo[:])

            if repeat == 1:
                for w in range(wpc):
                    window_body(w)
            else:
                def rbody(i):
                    for w in range(wpc):
                        window_body(w)
                with tc.For_i(0, repeat, 1) as _:
                    rbody(_)

    nc.compile()
    return nc


_PROG_CACHE = {}


def _get_prog(wpc, repeat=1, has_qkv_b=True, has_proj_b=True):
    key = (wpc, repeat, has_qkv_b, has_proj_b)
    if key not in _PROG_CACHE:
        _PROG_CACHE[key] = build_program(wpc, repeat, has_qkv_b, has_proj_b)
    return _PROG_CACHE[key]


def make_in_maps(inputs, wpc=WPC):
    ident, sigperm, ones = _host_consts()
    x = np.ascontiguousarray(np.asarray(inputs["x"], dtype=np.float32))
    shared = {
        "qkv_w": np.asarray(inputs["qkv_w"], np.float32),
        "qkv_b": np.asarray(inputs["qkv_b"], np.float32),
        "proj_w": np.asarray(inputs["proj_w"], np.float32),
        "proj_b": np.asarray(inputs["proj_b"], np.float32),
        "p1_w": np.asarray(inputs["p1_w"], np.float32),
        "p2_w": np.asarray(inputs["p2_w"], np.float32),
        "p3_w": np.asarray(inputs["p3_w"], np.float32),
        "p1_b": np.asarray(inputs["p1_b"], np.float32),
        "p2_b": np.asarray(inputs["p2_b"], np.float32),
        "p3_b": np.asarray(inputs["p3_b"], np.float32),
        "g1": np.asarray(inputs["g1"], np.float32),
        "g2": np.asarray(inputs["g2"], np.float32),
        "g3": np.asarray(inputs["g3"], np.float32),
        "b1": np.asarray(inputs["b1"], np.float32),
        "b2": np.asarray(inputs["b2"], np.float32),
        "b3": np.asarray(inputs["b3"], np.float32),
        "p4_w": np.asarray(inputs["p4_w"], np.float32),
        "p4_b": np.asarray(inputs["p4_b"], np.float32),
        "biases": np.asarray(inputs["biases"], np.float32),
        "ident": ident,
        "sigperm": sigperm,
        "ones": ones,
    }
    in_maps = []
    for cidx in range(NCORES):
        m = dict(shared)
        m["x"] = x[cidx * wpc:(cidx + 1) * wpc].reshape(wpc * N, C)
        in_maps.append(m)
    return in_maps


def kernel(**inputs):
    has_qkv_b = bool(np.any(np.asarray(inputs["qkv_b"])))
    has_proj_b = bool(np.any(np.asarray(inputs["proj_b"])))
    nc = _get_prog(WPC, 1, has_qkv_b, has_proj_b)
    in_maps = make_in_maps(inputs, WPC)
    res = run_bass_kernel_spmd(nc, in_maps, list(range(NCORES)))
    out = np.concatenate(
        [res.results[c]["y"].reshape(WPC, N, C) for c in range(NCORES)], axis=0
    )
    return out.astype(np.float32)


if __name__ == "__main__":
    rng = np.random.default_rng(0)
    demo = {
        "x": rng.standard_normal((B, N, C), dtype=np.float32),
    }
    print("use test.py for the full check")
